# revision 12
# baseline (speedup 1.0000x reference)
"""MixHop layer (gnn_message_passing) as a Trainium2 Bass/Tile SPMD kernel.

Math reformulation (A = sparse adjacency with edge weights, row=dst, col=src):
    x0 = x @ W0 + b0
    x1 = A @ (x @ W1 + b1) = (A @ x) @ W1 + d1 (x) b1      d1 = A @ 1
    x2 = A @ A @ (x @ W2 + b2) = (A @ A @ x) @ W2 + d2 (x) b2,  d2 = A @ d1
so only two sparse propagations of the raw features are needed:
    y1 = A @ x   (pass A),   y2 = A @ y1  (pass B, after all-gather of y1)

v3 design (driven by HW microbenchmarks):
  * dma_gather is descriptor-rate-bound (~5.3ns/desc shared wall), so pass A
    avoids gathers entirely: the host pre-expands x[col[e]] into dense bf16
    streams laid out partition-major; the device streams them with plain
    contiguous DMA.
  * pass B gathers y1 rows (256B bf16) from the all-gathered y1f.
  * All scatter matmuls run in bf16 (fp32 PSUM accumulate).  Selection
    matrices sel[e,r] = w[e]*(row[e]==r) are built per 128-edge tile on
    the Vector engine (tensor_scalar is_equal*mult) with a fraction
    offloaded to the idle Scalar engine as relu(-w*(iota-row)^2 + w).
  * Biases (1xb0, d1xb1, d2xb2) are K=1 rank-1 matmuls appended to the
    dense PSUM groups (PE has headroom; frees the Vector engine).
  * The AllGather of y1 (bf16) is chunked per stage-group and issued
    inline with pass A (pass A has no gpsimd work, so no head-of-line
    blocking); y1f is laid out stage-group-major so pass B's first
    source window is complete early.
  * pass B runs as two half-sweeps by source window: sweep 0 (first half
    of y1f) starts gathering while pass A is still finishing; per-block
    partial results are staged in SBUF and combined by per-block W2
    matmul groups at the end.
"""

import os
import sys

import numpy as np

for _p in ("/opt/trn_rl_repo",):
    if os.path.isdir(_p) and _p not in sys.path:
        sys.path.insert(0, _p)

import ml_dtypes

import concourse.bacc as bacc
import concourse.mybir as mybir
import concourse.tile as tile
from concourse.bass_utils import run_bass_kernel_spmd

F32 = mybir.dt.float32
BF16 = mybir.dt.bfloat16
I16 = mybir.dt.int16
NPBF = ml_dtypes.bfloat16

N_CORES = 8
P = 128          # partitions / rows per block / edges per tile
NCHUNK = 4       # source chunks (int16 index reach)
STAGE_TILE_CAP = 80   # max edge tiles staged in SBUF at once
SEL_ACT_EVERY = 4     # every k-th sel tile goes to the Scalar engine
AG_STAGES = 3         # stages per AllGather chunk


# ---------------------------------------------------------------------------
# host-side preprocessing
# ---------------------------------------------------------------------------

def _edge_grid(gblk, chunk, colrel, w, rloc, n_cores, B, nblk, nchunk):
    """Sort edges by (dest block, chunk); build the padded tile grid.

    Returns per-core tile grid metadata + per-edge slot assignment."""
    order = np.lexsort((chunk, gblk))
    gblk_s = gblk[order]
    chunk_s = chunk[order]
    col_s = colrel[order]
    w_s = w[order]
    rloc_s = rloc[order]

    E = len(gblk)
    grp = gblk_s * nchunk + chunk_s
    cnt = np.bincount(grp, minlength=nblk * nchunk)
    tiles = -(-cnt // P)
    T_BC = tiles.reshape(n_cores, B, nchunk).max(axis=0).astype(np.int64)
    empty = T_BC.sum(axis=1) == 0
    T_BC[empty, 0] = 1
    T_B = T_BC.sum(axis=1)
    LT = int(T_B.sum())

    stages = _make_stages(T_B)

    base_bc = np.zeros((B, nchunk), dtype=np.int64)
    pos0 = 0
    call_lens = []
    for (b0, nb, _) in stages:
        lens = []
        for c in range(nchunk):
            n = 0
            for b in range(b0, b0 + nb):
                base_bc[b, c] = pos0
                pos0 += T_BC[b, c]
                n += T_BC[b, c]
            lens.append(int(n))
        call_lens.append(lens)
    assert pos0 == LT

    grp_start = np.zeros(nblk * nchunk + 1, dtype=np.int64)
    np.cumsum(cnt, out=grp_start[1:])
    rank = np.arange(E, dtype=np.int64) - grp_start[grp]
    b_local = gblk_s % B
    e_core = gblk_s // B
    pos = P * base_bc[b_local, chunk_s] + rank

    L = P * LT
    emeta = np.zeros((n_cores, L, 4), dtype=np.float32)
    emeta[e_core, pos, 0] = rloc_s
    emeta[e_core, pos, 1] = -rloc_s
    emeta[e_core, pos, 2] = w_s
    emeta[e_core, pos, 3] = -w_s
    ecol = np.zeros((n_cores, L), dtype=np.int64)
    ecol[e_core, pos] = col_s
    used = np.zeros((n_cores, L), dtype=bool)
    used[e_core, pos] = True
    return dict(T_BC=T_BC, T_B=T_B, LT=LT, stages=stages,
                call_lens=call_lens, base_bc=base_bc, emeta=emeta,
                ecol=ecol, used=used)


def _prep(x, row, col, edge_weight, n_cores=N_CORES):
    N, C = x.shape
    S = -(-N // (n_cores * P)) * P          # shard rows per core
    NP = S * n_cores                        # padded node count
    B = S // P                              # dest blocks per core
    nblk = NP // P

    w64 = edge_weight.astype(np.float64)
    d1 = np.bincount(row, weights=w64, minlength=NP)[:NP]
    d2 = np.bincount(row, weights=w64 * d1[col], minlength=NP)[:NP]
    d1 = d1.astype(np.float32)
    d2 = d2.astype(np.float32)

    x16 = np.ascontiguousarray(x.astype(NPBF))

    gblk = (row // P).astype(np.int64)
    rloc = (row % P).astype(np.float32)
    w32 = edge_weight.astype(np.float32)
    col64 = col.astype(np.int64)

    # ---- pass A grid: no gathers -> no chunking at all ----
    gA = _edge_grid(gblk, np.zeros_like(gblk), col64, w32, rloc,
                    n_cores, B, nblk, 1)

    # dense edge-expanded features, partition-major bf16
    LT = gA["LT"]
    xeA = np.zeros((n_cores, P, LT * C), dtype=NPBF)
    for cix in range(n_cores):
        fe = x16[gA["ecol"][cix] % NP]
        fe[~gA["used"][cix]] = 0
        xeA[cix] = np.ascontiguousarray(
            fe.reshape(LT, P, C).transpose(1, 0, 2).reshape(P, LT * C))

    # ---- AllGather groups: NG block ranges, each flat size <= 32768 ----
    NG = min(NCHUNK, B)
    nb_g = [B // NG + (1 if g < B % NG else 0) for g in range(NG)]
    blk0_g = np.concatenate([[0], np.cumsum(nb_g)]).astype(np.int64)
    CHW = [n_cores * nb * P for nb in nb_g]      # flat rows per window
    assert all(ch <= 32768 for ch in CHW)
    flat_off = np.concatenate([[0], np.cumsum(CHW)]).astype(np.int64)
    assert flat_off[-1] == NP

    flat_block_off = np.zeros((n_cores, B), dtype=np.int64)
    for g in range(NG):
        b0 = int(blk0_g[g])
        for cix in range(n_cores):
            for b in range(b0, b0 + nb_g[g]):
                flat_block_off[cix, b] = (flat_off[g]
                                          + cix * nb_g[g] * P
                                          + (b - b0) * P)

    # ---- pass B grid: chunk = AllGather window of the source ----
    src_core = col64 // S
    src_b = (col64 % S) // P
    src_r = col64 % P
    flat_src = (flat_block_off[src_core, src_b] + src_r).astype(np.int64)
    gwin = np.searchsorted(flat_off[1:], flat_src, side="right")
    winrel = flat_src - flat_off[gwin]
    gB = _edge_grid(gblk, gwin, winrel, w32, rloc, n_cores, B, nblk, NG)
    LTB = gB["LT"]
    eidxB = gB["ecol"].astype(np.int16)
    eidxB16 = np.ascontiguousarray(
        np.tile(eidxB.reshape(n_cores, (P * LTB) // 16, 16)
                .transpose(0, 2, 1), (1, 8, 1)))

    x_pad16 = np.zeros((NP, C), dtype=NPBF)
    x_pad16[:N] = x16
    xT = np.ascontiguousarray(
        x_pad16.reshape(n_cores, S, C).transpose(0, 2, 1))

    d1_sb = np.ascontiguousarray(d1.reshape(n_cores, S))
    d2_sb = np.ascontiguousarray(d2.reshape(n_cores, S))

    ch_half = -(-NG // 2)          # sweep 0 = windows [0, ch_half)

    return dict(N=N, C=C, S=S, NP=NP, B=B, NG=NG, nb_g=nb_g,
                blk0_g=blk0_g, CHW=CHW, ch_half=ch_half,
                gA=gA, gB=gB, LT=LT, LTB=LTB,
                xeA=xeA, eidxB16=eidxB16, xT=xT, d1=d1_sb, d2=d2_sb)


def _make_stages(T_B, cap=STAGE_TILE_CAP):
    stages = []
    b = 0
    off = 0
    B = len(T_B)
    while b < B:
        start = b
        soff = off
        tot = 0
        while b < B and (b == start or tot + T_B[b] <= cap):
            tot += int(T_B[b])
            off += int(T_B[b])
            b += 1
        stages.append((start, b - start, soff))
    return stages


# ---------------------------------------------------------------------------
# device program
# ---------------------------------------------------------------------------

def build_program(meta, n_cores=N_CORES):
    C, S, NP, B = meta["C"], meta["S"], meta["NP"], meta["B"]
    NG, nb_g, blk0_g, CHW = (meta["NG"], meta["nb_g"], meta["blk0_g"],
                             meta["CHW"])
    ch_half = meta["ch_half"]
    gA, gB = meta["gA"], meta["gB"]
    LT, LTB = meta["LT"], meta["LTB"]
    stages = gA["stages"]
    stagesB = gB["stages"]
    ts_max = max(
        max(int(gA["T_B"][b0:b0 + nb].sum()) for b0, nb, _ in stages),
        max(int(gB["T_B"][b0:b0 + nb].sum()) for b0, nb, _ in stagesB))
    NBMAX = max(max(nb for _, nb, _ in stages),
                max(nb for _, nb, _ in stagesB))

    nc = bacc.Bacc("TRN2", target_bir_lowering=False, debug=False,
                   num_devices=n_cores, num_swdge_queues=4)

    xeA_d = nc.dram_tensor("xeA", [P, LT * C], BF16, kind="ExternalInput")
    emeta_d = nc.dram_tensor("emeta", [P, LT, 4], F32, kind="ExternalInput")
    emetaB_d = nc.dram_tensor("emetaB", [P, LTB, 4], F32,
                              kind="ExternalInput")
    eidxB_d = nc.dram_tensor("eidxB", [P, (P * LTB) // 16], I16,
                             kind="ExternalInput")
    xT_d = nc.dram_tensor("xT", [C, S], BF16, kind="ExternalInput")
    wmat_d = nc.dram_tensor("wmat", [C, 3 * C], BF16, kind="ExternalInput")
    consts_d = nc.dram_tensor("consts", [P, 2 * P], BF16,
                              kind="ExternalInput")
    consts32_d = nc.dram_tensor("consts32", [P, 3 * C + 2 * B], F32,
                                kind="ExternalInput")
    out_d = nc.dram_tensor("out", [S, 3 * C], F32, kind="ExternalOutput")

    sel_ctr = [0]

    with tile.TileContext(nc) as tc:
        with (
            tc.tile_pool(name="cpool", bufs=1) as cpool,
            tc.tile_pool(name="fpool", bufs=3) as fpool,
            tc.tile_pool(name="fpoolB", bufs=3) as fpoolB,
            tc.tile_pool(name="mpool", bufs=3) as mpool,
            tc.tile_pool(name="mpoolB", bufs=3) as mpoolB,
            tc.tile_pool(name="spool", bufs=10) as spool,
            tc.tile_pool(name="zpool", bufs=4) as zpool,
            tc.tile_pool(name="vpool", bufs=2) as vpool,
            tc.tile_pool(name="ypsum", bufs=3, space="PSUM") as ypsum,
            tc.tile_pool(name="tpsum", bufs=2, space="PSUM") as tpsum,
            tc.tile_pool(name="xpsum", bufs=3, space="PSUM") as xpsum,
            tc.tile_pool(name="dram2", bufs=1, space="DRAM") as dram,
        ):
            y1s = dram.tile([S, C], BF16)
            y1fg = []
            for g in range(NG):
                y1fg_g = dram.tile([CHW[g], C], BF16, addr_space="Shared",
                                   name=f"y1fg{g}")
                y1fg.append(y1fg_g)

            consts_t = cpool.tile([P, 2 * P], BF16, tag="consts")
            nc.sync.dma_start(consts_t[:], consts_d[:])
            iota_t = consts_t[:, 0 * P:1 * P]
            eye_t = consts_t[:, 1 * P:2 * P]
            consts32_t = cpool.tile([P, 3 * C + 2 * B], F32, tag="c32")
            nc.sync.dma_start(consts32_t[:], consts32_d[:])
            b0b = consts32_t[:, 0:C]
            b1b = consts32_t[:, C:2 * C]
            b2b = consts32_t[:, 2 * C:3 * C]
            d1col = consts32_t[:, 3 * C:3 * C + B]
            d2col = consts32_t[:, 3 * C + B:3 * C + 2 * B]
            wmat_t = cpool.tile([C, 3 * C], BF16, tag="wmat")
            nc.sync.dma_start(wmat_t[:], wmat_d[:])
            w0_t = wmat_t[:, 0 * C:1 * C]
            w1_t = wmat_t[:, 1 * C:2 * C]
            w2_t = wmat_t[:, 2 * C:3 * C]
            ystg = cpool.tile([P, B, C], BF16, tag="ystg")
            nc.vector.memset(ystg[:], 0.0)

            def make_sel(row_ap, nrow_ap, w_ap, nw_ap):
                sel_ctr[0] += 1
                sel = spool.tile([P, P], BF16, tag="sel")
                if sel_ctr[0] % SEL_ACT_EVERY == 0:
                    z2 = zpool.tile([P, P], BF16, tag="z2")
                    nc.scalar.activation(
                        out=z2[:], in_=iota_t,
                        func=mybir.ActivationFunctionType.Square,
                        bias=nrow_ap, scale=1.0)
                    nc.scalar.activation(
                        out=sel[:], in_=z2[:],
                        func=mybir.ActivationFunctionType.Relu,
                        bias=w_ap, scale=nw_ap)
                else:
                    nc.vector.tensor_scalar(
                        out=sel[:], in0=iota_t,
                        scalar1=row_ap, scalar2=w_ap,
                        op0=mybir.AluOpType.is_equal,
                        op1=mybir.AluOpType.mult)
                return sel

            # ---------------- pass A (+ inline chunked AllGather) --------
            def emit_ag(g):
                r0 = int(blk0_g[g]) * P
                rows = nb_g[g] * P
                nc.gpsimd.collective_compute(
                    "AllGather",
                    mybir.AluOpType.bypass,
                    replica_groups=[list(range(n_cores))],
                    ins=[y1s[r0:r0 + rows, :].opt()],
                    outs=[y1fg[g][:].opt()],
                )

            # AG windows 0..1 fire inline in pass A right after their
            # blocks complete; later windows are dispatched from inside
            # the pass-B sweeps (the gpsimd queue is in-order, so a late
            # AG would head-of-line-block the sweep gathers otherwise).
            n_inline = min(2, NG)
            ag_after_stage = {}
            for g in range(n_inline):
                lb = int(blk0_g[g + 1]) - 1
                for si, (b0, nb, _) in enumerate(stages):
                    if b0 <= lb < b0 + nb:
                        ag_after_stage.setdefault(si, []).append(g)
                        break

            for si, (b0, nb, soff) in enumerate(stages):
                ts = int(gA["T_B"][b0:b0 + nb].sum())
                mrow = mpool.tile([P, ts_max, 4], F32, tag="meta")
                nc.sync.dma_start(mrow[:, :ts, :],
                                  emeta_d[:, soff:soff + ts, :])
                f_t = fpool.tile([P, ts_max, C], BF16, tag="f")
                nc.sync.dma_start(
                    f_t[:, :ts, :],
                    xeA_d[:, soff * C:(soff + ts) * C]
                    .rearrange("p (t c) -> p t c", c=C))
                y_st = vpool.tile([P, NBMAX, C], BF16, tag="yst")
                for b in range(b0, b0 + nb):
                    kk = b - b0
                    y_ps = ypsum.tile([P, C], F32, tag="ypsum")
                    g0 = int(gA["base_bc"][b, 0] - soff)
                    ntile = int(gA["T_BC"][b, 0])
                    for k in range(ntile):
                        t = g0 + k
                        sel = make_sel(mrow[:, t, 0:1], mrow[:, t, 1:2],
                                       mrow[:, t, 2:3], mrow[:, t, 3:4])
                        nc.tensor.matmul(
                            out=y_ps[:], lhsT=sel[:], rhs=f_t[:, t, :],
                            start=(k == 0), stop=(k == ntile - 1))
                    nc.vector.tensor_copy(y_st[:, kk, :], y_ps[:])
                nc.sync.dma_start(
                    y1s[b0 * P:(b0 + nb) * P, :]
                    .rearrange("(g p) c -> p g c", p=P),
                    y_st[:, :nb, :])
                for g in ag_after_stage.get(si, []):
                    emit_ag(g)

            # ---------------- dense x0/x1 loops (overlap AG) -------------
            def emit_x0_x1(gsz=4):
                for g0 in range(0, B, gsz):
                    gn = min(gsz, B - g0)
                    xT_t = vpool.tile([C, gsz * P], BF16, tag="xT")
                    nc.sync.dma_start(xT_t[:, :gn * P],
                                      xT_d[:, g0 * P:(g0 + gn) * P])
                    y_ld = vpool.tile([P, gsz, C], BF16, tag="yld")
                    nc.sync.dma_start(
                        y_ld[:, :gn, :],
                        y1s[g0 * P:(g0 + gn) * P, :]
                        .rearrange("(g p) c -> p g c", p=P))
                    x01_st = vpool.tile([P, gsz, 2 * C], F32, tag="x01st")
                    for k in range(gn):
                        b = g0 + k
                        x0_ps = xpsum.tile([P, C], F32, tag="xpsum")
                        nc.tensor.matmul(
                            out=x0_ps[:], lhsT=xT_t[:, k * P:(k + 1) * P],
                            rhs=w0_t, start=True, stop=True)
                        nc.vector.tensor_tensor(
                            out=x01_st[:, k, 0:C], in0=x0_ps[:], in1=b0b,
                            op=mybir.AluOpType.add)
                        yT_ps = tpsum.tile([P, C], BF16, tag="tpsum")
                        nc.tensor.transpose(yT_ps[:], y_ld[:, k, :], eye_t)
                        yT_sb = vpool.tile([P, C], BF16, tag="ytsb")
                        nc.scalar.activation(
                            out=yT_sb[:], in_=yT_ps[:],
                            func=mybir.ActivationFunctionType.Copy,
                            scale=1.0)
                        x1_ps = xpsum.tile([P, C], F32, tag="xpsum")
                        nc.tensor.matmul(out=x1_ps[:], lhsT=yT_sb[:],
                                         rhs=w1_t, start=True, stop=True)
                        tmp1 = zpool.tile([P, C], F32, tag="tmp")
                        nc.scalar.activation(
                            out=tmp1[:], in_=b1b,
                            func=mybir.ActivationFunctionType.Copy,
                            scale=d1col[:, b:b + 1])
                        nc.vector.tensor_tensor(
                            out=x01_st[:, k, C:2 * C], in0=x1_ps[:],
                            in1=tmp1[:], op=mybir.AluOpType.add)
                    nc.sync.dma_start(
                        out_d[g0 * P:(g0 + gn) * P, 0:2 * C]
                        .rearrange("(g p) c -> p g c", p=P),
                        x01_st[:, :gn, :])

            emit_x0_x1()

            # ---------------- pass B: one sweep per source window ---------
            def emit_x2_blocks(b0, nb):
                x2_st = vpool.tile([P, NBMAX, C], F32, tag="x2st")
                for b in range(b0, b0 + nb):
                    kk = b - b0
                    x_ps = xpsum.tile([P, C], F32, tag="xpsum")
                    nc.tensor.matmul(out=x_ps[:], lhsT=ystg[:, b, :],
                                     rhs=w2_t, start=True, stop=True)
                    tmp2 = zpool.tile([P, C], F32, tag="tmp")
                    nc.scalar.activation(
                        out=tmp2[:], in_=b2b,
                        func=mybir.ActivationFunctionType.Copy,
                        scale=d2col[:, b:b + 1])
                    nc.vector.tensor_tensor(
                        out=x2_st[:, kk, :], in0=x_ps[:],
                        in1=tmp2[:], op=mybir.AluOpType.add)
                nc.sync.dma_start(
                    out_d[b0 * P:(b0 + nb) * P, 2 * C:3 * C]
                    .rearrange("(g p) c -> p g c", p=P),
                    x2_st[:, :nb, :])

            def emit_passB_sweep(sw):
                last = (sw == NG - 1)
                for si, (b0, nb, soff) in enumerate(stagesB):
                    # dispatch deferred AllGather windows mid-sweep
                    g_defer = sw + n_inline
                    if si == min(5, len(stagesB) - 1) and g_defer < NG:
                        emit_ag(g_defer)
                    lens = gB["call_lens"][si]
                    rel0 = sum(lens[:sw])
                    ts_sw = lens[sw]
                    if ts_sw == 0:
                        if last:
                            emit_x2_blocks(b0, nb)
                        continue
                    so = soff + rel0
                    mrow = mpoolB.tile([P, ts_max, 4], F32, tag="metaB")
                    nc.sync.dma_start(mrow[:, :ts_sw, :],
                                      emetaB_d[:, so:so + ts_sw, :])
                    idx_t = mpoolB.tile([P, (P * ts_max) // 16], I16,
                                        tag="idxB")
                    i16o = (P * so) // 16
                    i16n = (P * ts_sw) // 16
                    nc.sync.dma_start(idx_t[:, :i16n],
                                      eidxB_d[:, i16o:i16o + i16n])
                    f_t = fpoolB.tile([P, ts_max, C], BF16, tag="fB")
                    nidx = P * ts_sw
                    nc.gpsimd.dma_gather(
                        out_ap=f_t[:, :ts_sw, :],
                        in_ap=y1fg[sw][:, :],
                        idxs_ap=idx_t[:, :nidx // 16],
                        num_idxs=nidx,
                        num_idxs_reg=nidx,
                        elem_size=C,
                        single_packet=False,
                        queue_num=si % 4,
                    )
                    for b in range(b0, b0 + nb):
                        g0 = int(gB["base_bc"][b, sw] - so)
                        ntile = int(gB["T_BC"][b, sw])
                        if ntile == 0:
                            if last:
                                pass
                            continue
                        y_ps = ypsum.tile([P, C], F32, tag="ypsum")
                        for k in range(ntile):
                            t = g0 + k
                            sel = make_sel(mrow[:, t, 0:1],
                                           mrow[:, t, 1:2],
                                           mrow[:, t, 2:3],
                                           mrow[:, t, 3:4])
                            nc.tensor.matmul(
                                out=y_ps[:], lhsT=f_t[:, t, :],
                                rhs=sel[:],
                                start=(k == 0), stop=(k == ntile - 1))
                        ytmp = zpool.tile([P, C], BF16, tag="ytmp")
                        nc.scalar.activation(
                            out=ytmp[:], in_=y_ps[:],
                            func=mybir.ActivationFunctionType.Copy,
                            scale=1.0)
                        nc.vector.tensor_tensor(
                            out=ystg[:, b, :], in0=ytmp[:],
                            in1=ystg[:, b, :], op=mybir.AluOpType.add)
                    if last:
                        emit_x2_blocks(b0, nb)

            for sw in range(NG):
                emit_passB_sweep(sw)

    nc.compile()
    return nc


# ---------------------------------------------------------------------------
# entry point
# ---------------------------------------------------------------------------

def make_in_maps(meta, W0, b0, W1, b1, W2, b2, n_cores=N_CORES):
    C, S = meta["C"], meta["S"]
    iota = np.tile(np.arange(P, dtype=np.float32), (P, 1)).astype(NPBF)
    eye = np.eye(P, dtype=np.float32).astype(NPBF)
    consts = np.ascontiguousarray(np.concatenate([iota, eye], axis=1))
    wmat = np.concatenate(
        [np.asarray(W0, np.float32), np.asarray(W1, np.float32),
         np.asarray(W2, np.float32)], axis=1).astype(NPBF)
    B = meta["B"]
    b0b = np.tile(np.asarray(b0, np.float32), (P, 1))
    b1b = np.tile(np.asarray(b1, np.float32), (P, 1))
    b2b = np.tile(np.asarray(b2, np.float32), (P, 1))
    in_maps = []
    for c in range(n_cores):
        d1c = np.ascontiguousarray(
            meta["d1"][c].reshape(B, P).T)          # [128, B]
        d2c = np.ascontiguousarray(
            meta["d2"][c].reshape(B, P).T)
        consts32 = np.ascontiguousarray(np.concatenate(
            [b0b, b1b, b2b, d1c, d2c], axis=1))
        in_maps.append({
            "xeA": meta["xeA"][c],
            "emeta": _meta_tile(meta["gA"]["emeta"][c], meta["LT"]),
            "emetaB": _meta_tile(meta["gB"]["emeta"][c], meta["LTB"]),
            "eidxB": meta["eidxB16"][c],
            "xT": meta["xT"][c],
            "wmat": wmat,
            "consts": consts,
            "consts32": consts32,
        })
    return in_maps


def _meta_tile(em, LT_):
    # [L, 4] edge-order -> [128, LT, 4]: edge j -> partition j%128, tile j//128
    return np.ascontiguousarray(em.reshape(LT_, P, 4).transpose(1, 0, 2))


def kernel(x, row, col, edge_weight, W0, b0, W1, b1, W2, b2):
    x = np.asarray(x, np.float32)
    row = np.asarray(row, np.int32)
    col = np.asarray(col, np.int32)
    edge_weight = np.asarray(edge_weight, np.float32)
    N = x.shape[0]

    meta = _prep(x, row, col, edge_weight)
    nc = build_program(meta)
    in_maps = make_in_maps(meta, W0, b0, W1, b1, W2, b2)
    res = run_bass_kernel_spmd(nc, in_maps, core_ids=list(range(N_CORES)))
    out = np.concatenate([r["out"] for r in res.results], axis=0)
    return np.ascontiguousarray(out[:N])


if __name__ == "__main__":
    rng = np.random.default_rng(0)
    N, C, E = 2048, 128, 8192
    x = rng.standard_normal((N, C), dtype=np.float32)
    row = rng.integers(0, N, E).astype(np.int32)
    col = rng.integers(0, N, E).astype(np.int32)
    w = rng.random(E, dtype=np.float32)
    meta = _prep(x, row, col, w)
    print("tiles A:", meta["LT"], "tiles B:", meta["LTB"],
          "stages:", len(meta["gA"]["stages"]), len(meta["gB"]["stages"]))


# revision 13
# speedup vs baseline: 1.2079x; 1.2079x over previous
"""MixHop layer (gnn_message_passing) as a Trainium2 Bass/Tile SPMD kernel.

Math reformulation (A = sparse adjacency with edge weights, row=dst, col=src):
    x0 = x @ W0 + b0
    x1 = A @ (x @ W1 + b1) = (A @ x) @ W1 + d1 (x) b1      d1 = A @ 1
    x2 = A @ A @ (x @ W2 + b2) = (A @ A @ x) @ W2 + d2 (x) b2,  d2 = A @ d1
so only two sparse propagations of the raw features are needed:
    y1 = A @ x   (pass A),   y2 = A @ y1  (pass B, after all-gather of y1)

v3 design (driven by HW microbenchmarks):
  * dma_gather is descriptor-rate-bound (~5.3ns/desc shared wall), so pass A
    avoids gathers entirely: the host pre-expands x[col[e]] into dense bf16
    streams laid out partition-major; the device streams them with plain
    contiguous DMA.
  * pass B gathers y1 rows (256B bf16) from the all-gathered y1f.
  * All scatter matmuls run in bf16 (fp32 PSUM accumulate).  Selection
    matrices sel[e,r] = w[e]*(row[e]==r) are built per 128-edge tile on
    the Vector engine (tensor_scalar is_equal*mult) with a fraction
    offloaded to the idle Scalar engine as relu(-w*(iota-row)^2 + w).
  * Biases (1xb0, d1xb1, d2xb2) are K=1 rank-1 matmuls appended to the
    dense PSUM groups (PE has headroom; frees the Vector engine).
  * The AllGather of y1 (bf16) is chunked per stage-group and issued
    inline with pass A (pass A has no gpsimd work, so no head-of-line
    blocking); y1f is laid out stage-group-major so pass B's first
    source window is complete early.
  * pass B runs as two half-sweeps by source window: sweep 0 (first half
    of y1f) starts gathering while pass A is still finishing; per-block
    partial results are staged in SBUF and combined by per-block W2
    matmul groups at the end.
"""

import os
import sys

import numpy as np

for _p in ("/opt/trn_rl_repo",):
    if os.path.isdir(_p) and _p not in sys.path:
        sys.path.insert(0, _p)

import ml_dtypes

import concourse.bacc as bacc
import concourse.mybir as mybir
import concourse.tile as tile
from concourse.bass_utils import run_bass_kernel_spmd

F32 = mybir.dt.float32
BF16 = mybir.dt.bfloat16
I16 = mybir.dt.int16
NPBF = ml_dtypes.bfloat16

N_CORES = 8
P = 128          # partitions / rows per block / edges per tile
NCHUNK = 4       # source chunks (int16 index reach)
STAGE_TILE_CAP = 80   # max edge tiles staged in SBUF at once
SEL_ACT_EVERY = 4     # every k-th sel tile goes to the Scalar engine
AG_STAGES = 3         # stages per AllGather chunk


# ---------------------------------------------------------------------------
# host-side preprocessing
# ---------------------------------------------------------------------------

def _edge_grid(gblk, chunk, colrel, w, rloc, n_cores, B, nblk, nchunk):
    """Sort edges by (dest block, chunk); build the padded tile grid.

    Returns per-core tile grid metadata + per-edge slot assignment."""
    order = np.lexsort((chunk, gblk))
    gblk_s = gblk[order]
    chunk_s = chunk[order]
    col_s = colrel[order]
    w_s = w[order]
    rloc_s = rloc[order]

    E = len(gblk)
    grp = gblk_s * nchunk + chunk_s
    cnt = np.bincount(grp, minlength=nblk * nchunk)
    tiles = -(-cnt // P)
    T_BC = tiles.reshape(n_cores, B, nchunk).max(axis=0).astype(np.int64)
    empty = T_BC.sum(axis=1) == 0
    T_BC[empty, 0] = 1
    T_B = T_BC.sum(axis=1)
    LT = int(T_B.sum())

    stages = _make_stages(T_B)

    base_bc = np.zeros((B, nchunk), dtype=np.int64)
    pos0 = 0
    call_lens = []
    for (b0, nb, _) in stages:
        lens = []
        for c in range(nchunk):
            n = 0
            for b in range(b0, b0 + nb):
                base_bc[b, c] = pos0
                pos0 += T_BC[b, c]
                n += T_BC[b, c]
            lens.append(int(n))
        call_lens.append(lens)
    assert pos0 == LT

    grp_start = np.zeros(nblk * nchunk + 1, dtype=np.int64)
    np.cumsum(cnt, out=grp_start[1:])
    rank = np.arange(E, dtype=np.int64) - grp_start[grp]
    b_local = gblk_s % B
    e_core = gblk_s // B
    pos = P * base_bc[b_local, chunk_s] + rank

    L = P * LT
    emeta = np.zeros((n_cores, L, 4), dtype=np.float32)
    emeta[e_core, pos, 0] = rloc_s
    emeta[e_core, pos, 1] = -rloc_s
    emeta[e_core, pos, 2] = w_s
    emeta[e_core, pos, 3] = -w_s
    ecol = np.zeros((n_cores, L), dtype=np.int64)
    ecol[e_core, pos] = col_s
    used = np.zeros((n_cores, L), dtype=bool)
    used[e_core, pos] = True
    return dict(T_BC=T_BC, T_B=T_B, LT=LT, stages=stages,
                call_lens=call_lens, base_bc=base_bc, emeta=emeta,
                ecol=ecol, used=used)


def _prep(x, row, col, edge_weight, n_cores=N_CORES):
    N, C = x.shape
    S = -(-N // (n_cores * P)) * P          # shard rows per core
    NP = S * n_cores                        # padded node count
    B = S // P                              # dest blocks per core
    nblk = NP // P

    w64 = edge_weight.astype(np.float64)
    d1 = np.bincount(row, weights=w64, minlength=NP)[:NP]
    d2 = np.bincount(row, weights=w64 * d1[col], minlength=NP)[:NP]
    d1 = d1.astype(np.float32)
    d2 = d2.astype(np.float32)

    x16 = np.ascontiguousarray(x.astype(NPBF))

    gblk = (row // P).astype(np.int64)
    rloc = (row % P).astype(np.float32)
    w32 = edge_weight.astype(np.float32)
    col64 = col.astype(np.int64)

    # ---- pass A grid: no gathers -> no chunking at all ----
    gA = _edge_grid(gblk, np.zeros_like(gblk), col64, w32, rloc,
                    n_cores, B, nblk, 1)

    # dense edge-expanded features, partition-major bf16
    LT = gA["LT"]
    xeA = np.zeros((n_cores, P, LT * C), dtype=NPBF)
    for cix in range(n_cores):
        fe = x16[gA["ecol"][cix] % NP]
        fe[~gA["used"][cix]] = 0
        xeA[cix] = np.ascontiguousarray(
            fe.reshape(LT, P, C).transpose(1, 0, 2).reshape(P, LT * C))

    # ---- AllGather groups: NG block ranges, each flat size <= 32768 ----
    NG = min(NCHUNK, B)
    nb_g = [B // NG + (1 if g < B % NG else 0) for g in range(NG)]
    blk0_g = np.concatenate([[0], np.cumsum(nb_g)]).astype(np.int64)
    CHW = [n_cores * nb * P for nb in nb_g]      # flat rows per window
    assert all(ch <= 32768 for ch in CHW)
    flat_off = np.concatenate([[0], np.cumsum(CHW)]).astype(np.int64)
    assert flat_off[-1] == NP

    flat_block_off = np.zeros((n_cores, B), dtype=np.int64)
    for g in range(NG):
        b0 = int(blk0_g[g])
        for cix in range(n_cores):
            for b in range(b0, b0 + nb_g[g]):
                flat_block_off[cix, b] = (flat_off[g]
                                          + cix * nb_g[g] * P
                                          + (b - b0) * P)

    # ---- pass B grid: chunk = AllGather window of the source ----
    src_core = col64 // S
    src_b = (col64 % S) // P
    src_r = col64 % P
    flat_src = (flat_block_off[src_core, src_b] + src_r).astype(np.int64)
    gwin = np.searchsorted(flat_off[1:], flat_src, side="right")
    winrel = flat_src - flat_off[gwin]
    gB = _edge_grid(gblk, gwin, winrel, w32, rloc, n_cores, B, nblk, NG)
    LTB = gB["LT"]
    eidxB = gB["ecol"].astype(np.int16)
    eidxB16 = np.ascontiguousarray(
        np.tile(eidxB.reshape(n_cores, (P * LTB) // 16, 16)
                .transpose(0, 2, 1), (1, 8, 1)))

    x_pad16 = np.zeros((NP, C), dtype=NPBF)
    x_pad16[:N] = x16
    xT = np.ascontiguousarray(
        x_pad16.reshape(n_cores, S, C).transpose(0, 2, 1))

    d1_sb = np.ascontiguousarray(d1.reshape(n_cores, S))
    d2_sb = np.ascontiguousarray(d2.reshape(n_cores, S))

    ch_half = -(-NG // 2)          # sweep 0 = windows [0, ch_half)

    return dict(N=N, C=C, S=S, NP=NP, B=B, NG=NG, nb_g=nb_g,
                blk0_g=blk0_g, CHW=CHW, ch_half=ch_half,
                gA=gA, gB=gB, LT=LT, LTB=LTB,
                xeA=xeA, eidxB16=eidxB16, xT=xT, d1=d1_sb, d2=d2_sb)


def _make_stages(T_B, cap=STAGE_TILE_CAP):
    stages = []
    b = 0
    off = 0
    B = len(T_B)
    while b < B:
        start = b
        soff = off
        tot = 0
        while b < B and (b == start or tot + T_B[b] <= cap):
            tot += int(T_B[b])
            off += int(T_B[b])
            b += 1
        stages.append((start, b - start, soff))
    return stages


# ---------------------------------------------------------------------------
# device program
# ---------------------------------------------------------------------------

def build_program(meta, n_cores=N_CORES):
    C, S, NP, B = meta["C"], meta["S"], meta["NP"], meta["B"]
    NG, nb_g, blk0_g, CHW = (meta["NG"], meta["nb_g"], meta["blk0_g"],
                             meta["CHW"])
    ch_half = meta["ch_half"]
    gA, gB = meta["gA"], meta["gB"]
    LT, LTB = meta["LT"], meta["LTB"]
    stages = gA["stages"]
    stagesB = gB["stages"]
    ts_max = max(
        max(int(gA["T_B"][b0:b0 + nb].sum()) for b0, nb, _ in stages),
        max(int(gB["T_B"][b0:b0 + nb].sum()) for b0, nb, _ in stagesB))
    NBMAX = max(max(nb for _, nb, _ in stages),
                max(nb for _, nb, _ in stagesB))

    nc = bacc.Bacc("TRN2", target_bir_lowering=False, debug=False,
                   num_devices=n_cores, num_swdge_queues=4)

    xeA_d = nc.dram_tensor("xeA", [P, LT * C], BF16, kind="ExternalInput")
    emeta_d = nc.dram_tensor("emeta", [P, LT, 4], F32, kind="ExternalInput")
    emetaB_d = nc.dram_tensor("emetaB", [P, LTB, 4], F32,
                              kind="ExternalInput")
    eidxB_d = nc.dram_tensor("eidxB", [P, (P * LTB) // 16], I16,
                             kind="ExternalInput")
    xT_d = nc.dram_tensor("xT", [C, S], BF16, kind="ExternalInput")
    wmat_d = nc.dram_tensor("wmat", [C, 3 * C], BF16, kind="ExternalInput")
    consts_d = nc.dram_tensor("consts", [P, 2 * P], BF16,
                              kind="ExternalInput")
    consts32_d = nc.dram_tensor("consts32", [P, 3 * C + 2 * B], F32,
                                kind="ExternalInput")
    out_d = nc.dram_tensor("out", [S, 3 * C], F32, kind="ExternalOutput")

    sel_ctr = [0]

    with tile.TileContext(nc) as tc:
        with (
            tc.tile_pool(name="cpool", bufs=1) as cpool,
            tc.tile_pool(name="fpool", bufs=3) as fpool,
            tc.tile_pool(name="fpoolB", bufs=3) as fpoolB,
            tc.tile_pool(name="mpool", bufs=3) as mpool,
            tc.tile_pool(name="mpoolB", bufs=3) as mpoolB,
            tc.tile_pool(name="spool", bufs=10) as spool,
            tc.tile_pool(name="zpool", bufs=4) as zpool,
            tc.tile_pool(name="vpool", bufs=2) as vpool,
            tc.tile_pool(name="ypsum", bufs=3, space="PSUM") as ypsum,
            tc.tile_pool(name="tpsum", bufs=2, space="PSUM") as tpsum,
            tc.tile_pool(name="xpsum", bufs=3, space="PSUM") as xpsum,
            tc.tile_pool(name="dram2", bufs=1, space="DRAM") as dram,
        ):
            y1s = dram.tile([S, C], BF16)
            y1fg = []
            for g in range(NG):
                y1fg_g = dram.tile([CHW[g], C], BF16, addr_space="Shared",
                                   name=f"y1fg{g}")
                y1fg.append(y1fg_g)

            consts_t = cpool.tile([P, 2 * P], BF16, tag="consts")
            nc.sync.dma_start(consts_t[:], consts_d[:])
            iota_t = consts_t[:, 0 * P:1 * P]
            eye_t = consts_t[:, 1 * P:2 * P]
            consts32_t = cpool.tile([P, 3 * C + 2 * B], F32, tag="c32")
            nc.sync.dma_start(consts32_t[:], consts32_d[:])
            b0b = consts32_t[:, 0:C]
            b1b = consts32_t[:, C:2 * C]
            b2b = consts32_t[:, 2 * C:3 * C]
            d1col = consts32_t[:, 3 * C:3 * C + B]
            d2col = consts32_t[:, 3 * C + B:3 * C + 2 * B]
            wmat_t = cpool.tile([C, 3 * C], BF16, tag="wmat")
            nc.sync.dma_start(wmat_t[:], wmat_d[:])
            w0_t = wmat_t[:, 0 * C:1 * C]
            w1_t = wmat_t[:, 1 * C:2 * C]
            w2_t = wmat_t[:, 2 * C:3 * C]
            ystg = cpool.tile([P, B, C], BF16, tag="ystg")
            nc.vector.memset(ystg[:], 0.0)

            def make_sel(row_ap, nrow_ap, w_ap, nw_ap):
                sel_ctr[0] += 1
                sel = spool.tile([P, P], BF16, tag="sel")
                if sel_ctr[0] % SEL_ACT_EVERY == 0:
                    z2 = zpool.tile([P, P], BF16, tag="z2")
                    nc.scalar.activation(
                        out=z2[:], in_=iota_t,
                        func=mybir.ActivationFunctionType.Square,
                        bias=nrow_ap, scale=1.0)
                    nc.scalar.activation(
                        out=sel[:], in_=z2[:],
                        func=mybir.ActivationFunctionType.Relu,
                        bias=w_ap, scale=nw_ap)
                else:
                    nc.vector.tensor_scalar(
                        out=sel[:], in0=iota_t,
                        scalar1=row_ap, scalar2=w_ap,
                        op0=mybir.AluOpType.is_equal,
                        op1=mybir.AluOpType.mult)
                return sel

            # ---------------- pass A (+ inline chunked AllGather) --------
            def emit_ag(g):
                r0 = int(blk0_g[g]) * P
                rows = nb_g[g] * P
                nc.gpsimd.collective_compute(
                    "AllGather",
                    mybir.AluOpType.bypass,
                    replica_groups=[list(range(n_cores))],
                    ins=[y1s[r0:r0 + rows, :].opt()],
                    outs=[y1fg[g][:].opt()],
                )

            # AG windows 0..1 fire inline in pass A right after their
            # blocks complete; later windows are dispatched from inside
            # the pass-B sweeps (the gpsimd queue is in-order, so a late
            # AG would head-of-line-block the sweep gathers otherwise).
            n_inline = min(2, NG)
            ag_after_stage = {}
            for g in range(n_inline):
                lb = int(blk0_g[g + 1]) - 1
                for si, (b0, nb, _) in enumerate(stages):
                    if b0 <= lb < b0 + nb:
                        ag_after_stage.setdefault(si, []).append(g)
                        break

            for si, (b0, nb, soff) in enumerate(stages):
                ts = int(gA["T_B"][b0:b0 + nb].sum())
                mrow = mpool.tile([P, ts_max, 4], F32, tag="meta")
                nc.sync.dma_start(mrow[:, :ts, :],
                                  emeta_d[:, soff:soff + ts, :])
                f_t = fpool.tile([P, ts_max, C], BF16, tag="f")
                nc.sync.dma_start(
                    f_t[:, :ts, :],
                    xeA_d[:, soff * C:(soff + ts) * C]
                    .rearrange("p (t c) -> p t c", c=C))
                y_st = vpool.tile([P, NBMAX, C], BF16, tag="yst")
                for b in range(b0, b0 + nb):
                    kk = b - b0
                    y_ps = ypsum.tile([P, C], F32, tag="ypsum")
                    g0 = int(gA["base_bc"][b, 0] - soff)
                    ntile = int(gA["T_BC"][b, 0])
                    for k in range(ntile):
                        t = g0 + k
                        sel = make_sel(mrow[:, t, 0:1], mrow[:, t, 1:2],
                                       mrow[:, t, 2:3], mrow[:, t, 3:4])
                        nc.tensor.matmul(
                            out=y_ps[:], lhsT=sel[:], rhs=f_t[:, t, :],
                            start=(k == 0), stop=(k == ntile - 1))
                    nc.vector.tensor_copy(y_st[:, kk, :], y_ps[:])
                nc.sync.dma_start(
                    y1s[b0 * P:(b0 + nb) * P, :]
                    .rearrange("(g p) c -> p g c", p=P),
                    y_st[:, :nb, :])
                for g in ag_after_stage.get(si, []):
                    emit_ag(g)

            # ---------------- dense x0/x1 loops (overlap AG) -------------
            def emit_x0_x1(gsz=4):
                for g0 in range(0, B, gsz):
                    gn = min(gsz, B - g0)
                    xT_t = vpool.tile([C, gsz * P], BF16, tag="xT")
                    nc.sync.dma_start(xT_t[:, :gn * P],
                                      xT_d[:, g0 * P:(g0 + gn) * P])
                    y_ld = vpool.tile([P, gsz, C], BF16, tag="yld")
                    nc.sync.dma_start(
                        y_ld[:, :gn, :],
                        y1s[g0 * P:(g0 + gn) * P, :]
                        .rearrange("(g p) c -> p g c", p=P))
                    x01_st = vpool.tile([P, gsz, 2 * C], F32, tag="x01st")
                    for k in range(gn):
                        b = g0 + k
                        x0_ps = xpsum.tile([P, C], F32, tag="xpsum")
                        nc.tensor.matmul(
                            out=x0_ps[:], lhsT=xT_t[:, k * P:(k + 1) * P],
                            rhs=w0_t, start=True, stop=True)
                        nc.vector.tensor_tensor(
                            out=x01_st[:, k, 0:C], in0=x0_ps[:], in1=b0b,
                            op=mybir.AluOpType.add)
                        yT_ps = tpsum.tile([P, C], BF16, tag="tpsum")
                        nc.tensor.transpose(yT_ps[:], y_ld[:, k, :], eye_t)
                        yT_sb = vpool.tile([P, C], BF16, tag="ytsb")
                        nc.scalar.activation(
                            out=yT_sb[:], in_=yT_ps[:],
                            func=mybir.ActivationFunctionType.Copy,
                            scale=1.0)
                        x1_ps = xpsum.tile([P, C], F32, tag="xpsum")
                        nc.tensor.matmul(out=x1_ps[:], lhsT=yT_sb[:],
                                         rhs=w1_t, start=True, stop=True)
                        tmp1 = zpool.tile([P, C], F32, tag="tmp")
                        nc.scalar.activation(
                            out=tmp1[:], in_=b1b,
                            func=mybir.ActivationFunctionType.Copy,
                            scale=d1col[:, b:b + 1])
                        nc.vector.tensor_tensor(
                            out=x01_st[:, k, C:2 * C], in0=x1_ps[:],
                            in1=tmp1[:], op=mybir.AluOpType.add)
                    nc.sync.dma_start(
                        out_d[g0 * P:(g0 + gn) * P, 0:2 * C]
                        .rearrange("(g p) c -> p g c", p=P),
                        x01_st[:, :gn, :])

            emit_x0_x1()

            # ---------------- pass B: one sweep per source window ---------
            def emit_x2_blocks(b0, nb):
                x2_st = vpool.tile([P, NBMAX, C], F32, tag="x2st")
                for b in range(b0, b0 + nb):
                    kk = b - b0
                    x_ps = xpsum.tile([P, C], F32, tag="xpsum")
                    nc.tensor.matmul(out=x_ps[:], lhsT=ystg[:, b, :],
                                     rhs=w2_t, start=True, stop=True)
                    tmp2 = zpool.tile([P, C], F32, tag="tmp")
                    nc.scalar.activation(
                        out=tmp2[:], in_=b2b,
                        func=mybir.ActivationFunctionType.Copy,
                        scale=d2col[:, b:b + 1])
                    nc.vector.tensor_tensor(
                        out=x2_st[:, kk, :], in0=x_ps[:],
                        in1=tmp2[:], op=mybir.AluOpType.add)
                nc.sync.dma_start(
                    out_d[b0 * P:(b0 + nb) * P, 2 * C:3 * C]
                    .rearrange("(g p) c -> p g c", p=P),
                    x2_st[:, :nb, :])

            def emit_passB_sweep(sw):
                clo = 0 if sw == 0 else ch_half
                chi = ch_half if sw == 0 else NG
                last = (sw == 1)
                for si, (b0, nb, soff) in enumerate(stagesB):
                    # dispatch deferred AllGather windows early in sweep 0
                    if sw == 0 and si in (3, 6):
                        g_defer = n_inline + (0 if si == 3 else 1)
                        if g_defer < NG and si == 3:
                            for g in range(n_inline, min(n_inline + 1, NG)):
                                emit_ag(g)
                        elif si == 6:
                            for g in range(n_inline + 1, NG):
                                emit_ag(g)
                    lens = gB["call_lens"][si]
                    rel0 = sum(lens[:clo])
                    ts_sw = sum(lens[clo:chi])
                    if ts_sw == 0:
                        if last:
                            emit_x2_blocks(b0, nb)
                        continue
                    so = soff + rel0
                    mrow = mpoolB.tile([P, ts_max, 4], F32, tag="metaB")
                    nc.sync.dma_start(mrow[:, :ts_sw, :],
                                      emetaB_d[:, so:so + ts_sw, :])
                    idx_t = mpoolB.tile([P, (P * ts_max) // 16], I16,
                                        tag="idxB")
                    i16o = (P * so) // 16
                    i16n = (P * ts_sw) // 16
                    nc.sync.dma_start(idx_t[:, :i16n],
                                      eidxB_d[:, i16o:i16o + i16n])
                    f_t = fpoolB.tile([P, ts_max, C], BF16, tag="fB")
                    rel = 0
                    for c in range(clo, chi):
                        tsc = lens[c]
                        if tsc == 0:
                            continue
                        nidx = P * tsc
                        nc.gpsimd.dma_gather(
                            out_ap=f_t[:, rel:rel + tsc, :],
                            in_ap=y1fg[c][:, :],
                            idxs_ap=idx_t[:, (P * rel) // 16:
                                          (P * rel) // 16 + nidx // 16],
                            num_idxs=nidx,
                            num_idxs_reg=nidx,
                            elem_size=C,
                            single_packet=False,
                            queue_num=(si + c) % 4,
                        )
                        rel += tsc
                    for b in range(b0, b0 + nb):
                        tl = [(int(gB["base_bc"][b, c] - so),
                               int(gB["T_BC"][b, c]))
                              for c in range(clo, chi)
                              if gB["T_BC"][b, c] > 0]
                        ntile = sum(n for _, n in tl)
                        if ntile == 0:
                            continue
                        y_ps = ypsum.tile([P, C], F32, tag="ypsum")
                        k = 0
                        for (g0, n) in tl:
                            for t in range(g0, g0 + n):
                                sel = make_sel(mrow[:, t, 0:1],
                                               mrow[:, t, 1:2],
                                               mrow[:, t, 2:3],
                                               mrow[:, t, 3:4])
                                nc.tensor.matmul(
                                    out=y_ps[:], lhsT=f_t[:, t, :],
                                    rhs=sel[:],
                                    start=(k == 0), stop=(k == ntile - 1))
                                k += 1
                        if sw == 0:
                            nc.scalar.activation(
                                out=ystg[:, b, :], in_=y_ps[:],
                                func=mybir.ActivationFunctionType.Copy,
                                scale=1.0)
                        else:
                            ytmp = zpool.tile([P, C], BF16, tag="ytmp")
                            nc.scalar.activation(
                                out=ytmp[:], in_=y_ps[:],
                                func=mybir.ActivationFunctionType.Copy,
                                scale=1.0)
                            nc.vector.tensor_tensor(
                                out=ystg[:, b, :], in0=ytmp[:],
                                in1=ystg[:, b, :], op=mybir.AluOpType.add)
                    if last:
                        emit_x2_blocks(b0, nb)

            emit_passB_sweep(0)
            emit_passB_sweep(1)

    nc.compile()
    return nc


# ---------------------------------------------------------------------------
# entry point
# ---------------------------------------------------------------------------

def make_in_maps(meta, W0, b0, W1, b1, W2, b2, n_cores=N_CORES):
    C, S = meta["C"], meta["S"]
    iota = np.tile(np.arange(P, dtype=np.float32), (P, 1)).astype(NPBF)
    eye = np.eye(P, dtype=np.float32).astype(NPBF)
    consts = np.ascontiguousarray(np.concatenate([iota, eye], axis=1))
    wmat = np.concatenate(
        [np.asarray(W0, np.float32), np.asarray(W1, np.float32),
         np.asarray(W2, np.float32)], axis=1).astype(NPBF)
    B = meta["B"]
    b0b = np.tile(np.asarray(b0, np.float32), (P, 1))
    b1b = np.tile(np.asarray(b1, np.float32), (P, 1))
    b2b = np.tile(np.asarray(b2, np.float32), (P, 1))
    in_maps = []
    for c in range(n_cores):
        d1c = np.ascontiguousarray(
            meta["d1"][c].reshape(B, P).T)          # [128, B]
        d2c = np.ascontiguousarray(
            meta["d2"][c].reshape(B, P).T)
        consts32 = np.ascontiguousarray(np.concatenate(
            [b0b, b1b, b2b, d1c, d2c], axis=1))
        in_maps.append({
            "xeA": meta["xeA"][c],
            "emeta": _meta_tile(meta["gA"]["emeta"][c], meta["LT"]),
            "emetaB": _meta_tile(meta["gB"]["emeta"][c], meta["LTB"]),
            "eidxB": meta["eidxB16"][c],
            "xT": meta["xT"][c],
            "wmat": wmat,
            "consts": consts,
            "consts32": consts32,
        })
    return in_maps


def _meta_tile(em, LT_):
    # [L, 4] edge-order -> [128, LT, 4]: edge j -> partition j%128, tile j//128
    return np.ascontiguousarray(em.reshape(LT_, P, 4).transpose(1, 0, 2))


def kernel(x, row, col, edge_weight, W0, b0, W1, b1, W2, b2):
    x = np.asarray(x, np.float32)
    row = np.asarray(row, np.int32)
    col = np.asarray(col, np.int32)
    edge_weight = np.asarray(edge_weight, np.float32)
    N = x.shape[0]

    meta = _prep(x, row, col, edge_weight)
    nc = build_program(meta)
    in_maps = make_in_maps(meta, W0, b0, W1, b1, W2, b2)
    res = run_bass_kernel_spmd(nc, in_maps, core_ids=list(range(N_CORES)))
    out = np.concatenate([r["out"] for r in res.results], axis=0)
    return np.ascontiguousarray(out[:N])


if __name__ == "__main__":
    rng = np.random.default_rng(0)
    N, C, E = 2048, 128, 8192
    x = rng.standard_normal((N, C), dtype=np.float32)
    row = rng.integers(0, N, E).astype(np.int32)
    col = rng.integers(0, N, E).astype(np.int32)
    w = rng.random(E, dtype=np.float32)
    meta = _prep(x, row, col, w)
    print("tiles A:", meta["LT"], "tiles B:", meta["LTB"],
          "stages:", len(meta["gA"]["stages"]), len(meta["gB"]["stages"]))


# revision 14
# speedup vs baseline: 1.4302x; 1.1840x over previous
"""MixHop layer (gnn_message_passing) as a Trainium2 Bass/Tile SPMD kernel.

Math reformulation (A = sparse adjacency with edge weights, row=dst, col=src):
    x0 = x @ W0 + b0
    x1 = A @ (x @ W1 + b1) = (A @ x) @ W1 + d1 (x) b1      d1 = A @ 1
    x2 = A @ A @ (x @ W2 + b2) = (A @ A @ x) @ W2 + d2 (x) b2,  d2 = A @ d1
so only two sparse propagations of the raw features are needed:
    y1 = A @ x   (pass A),   y2 = A @ y1  (pass B, after all-gather of y1)

v3 design (driven by HW microbenchmarks):
  * dma_gather is descriptor-rate-bound (~5.3ns/desc shared wall), so pass A
    avoids gathers entirely: the host pre-expands x[col[e]] into dense bf16
    streams laid out partition-major; the device streams them with plain
    contiguous DMA.
  * pass B gathers y1 rows (256B bf16) from the all-gathered y1f.
  * All scatter matmuls run in bf16 (fp32 PSUM accumulate).  Selection
    matrices sel[e,r] = w[e]*(row[e]==r) are built per 128-edge tile on
    the Vector engine (tensor_scalar is_equal*mult) with a fraction
    offloaded to the idle Scalar engine as relu(-w*(iota-row)^2 + w).
  * Biases (1xb0, d1xb1, d2xb2) are K=1 rank-1 matmuls appended to the
    dense PSUM groups (PE has headroom; frees the Vector engine).
  * The AllGather of y1 (bf16) is chunked per stage-group and issued
    inline with pass A (pass A has no gpsimd work, so no head-of-line
    blocking); y1f is laid out stage-group-major so pass B's first
    source window is complete early.
  * pass B runs as two half-sweeps by source window: sweep 0 (first half
    of y1f) starts gathering while pass A is still finishing; per-block
    partial results are staged in SBUF and combined by per-block W2
    matmul groups at the end.
"""

import os
import sys

import numpy as np

for _p in ("/opt/trn_rl_repo",):
    if os.path.isdir(_p) and _p not in sys.path:
        sys.path.insert(0, _p)

import ml_dtypes

import concourse.bacc as bacc
import concourse.mybir as mybir
import concourse.tile as tile
from concourse.bass_utils import run_bass_kernel_spmd

F32 = mybir.dt.float32
BF16 = mybir.dt.bfloat16
I16 = mybir.dt.int16
NPBF = ml_dtypes.bfloat16

N_CORES = 8
P = 128          # partitions / rows per block / edges per tile
NCHUNK = 4       # source chunks (int16 index reach)
STAGE_TILE_CAP = 80   # max edge tiles staged in SBUF at once
SEL_ACT_EVERY = 4     # every k-th sel tile goes to the Scalar engine
AG_STAGES = 3         # stages per AllGather chunk


# ---------------------------------------------------------------------------
# host-side preprocessing
# ---------------------------------------------------------------------------

def _edge_grid(gblk, chunk, colrel, w, rloc, n_cores, B, nblk, nchunk):
    """Sort edges by (dest block, chunk); build the padded tile grid.

    Returns per-core tile grid metadata + per-edge slot assignment."""
    order = np.lexsort((chunk, gblk))
    gblk_s = gblk[order]
    chunk_s = chunk[order]
    col_s = colrel[order]
    w_s = w[order]
    rloc_s = rloc[order]

    E = len(gblk)
    grp = gblk_s * nchunk + chunk_s
    cnt = np.bincount(grp, minlength=nblk * nchunk)
    tiles = -(-cnt // P)
    T_BC = tiles.reshape(n_cores, B, nchunk).max(axis=0).astype(np.int64)
    empty = T_BC.sum(axis=1) == 0
    T_BC[empty, 0] = 1
    T_B = T_BC.sum(axis=1)
    LT = int(T_B.sum())

    stages = _make_stages(T_B)

    base_bc = np.zeros((B, nchunk), dtype=np.int64)
    pos0 = 0
    call_lens = []
    for (b0, nb, _) in stages:
        lens = []
        for c in range(nchunk):
            n = 0
            for b in range(b0, b0 + nb):
                base_bc[b, c] = pos0
                pos0 += T_BC[b, c]
                n += T_BC[b, c]
            lens.append(int(n))
        call_lens.append(lens)
    assert pos0 == LT

    grp_start = np.zeros(nblk * nchunk + 1, dtype=np.int64)
    np.cumsum(cnt, out=grp_start[1:])
    rank = np.arange(E, dtype=np.int64) - grp_start[grp]
    b_local = gblk_s % B
    e_core = gblk_s // B
    pos = P * base_bc[b_local, chunk_s] + rank

    L = P * LT
    emeta = np.zeros((n_cores, L, 4), dtype=np.float32)
    emeta[e_core, pos, 0] = rloc_s
    emeta[e_core, pos, 1] = -rloc_s
    emeta[e_core, pos, 2] = w_s
    emeta[e_core, pos, 3] = -w_s
    ecol = np.zeros((n_cores, L), dtype=np.int64)
    ecol[e_core, pos] = col_s
    used = np.zeros((n_cores, L), dtype=bool)
    used[e_core, pos] = True
    return dict(T_BC=T_BC, T_B=T_B, LT=LT, stages=stages,
                call_lens=call_lens, base_bc=base_bc, emeta=emeta,
                ecol=ecol, used=used)


def _prep(x, row, col, edge_weight, n_cores=N_CORES):
    N, C = x.shape
    S = -(-N // (n_cores * P)) * P          # shard rows per core
    NP = S * n_cores                        # padded node count
    B = S // P                              # dest blocks per core
    nblk = NP // P

    w64 = edge_weight.astype(np.float64)
    d1 = np.bincount(row, weights=w64, minlength=NP)[:NP]
    d2 = np.bincount(row, weights=w64 * d1[col], minlength=NP)[:NP]
    d1 = d1.astype(np.float32)
    d2 = d2.astype(np.float32)

    x16 = np.ascontiguousarray(x.astype(NPBF))

    gblk = (row // P).astype(np.int64)
    rloc = (row % P).astype(np.float32)
    w32 = edge_weight.astype(np.float32)
    col64 = col.astype(np.int64)

    # ---- pass A grid: no gathers -> no chunking at all ----
    gA = _edge_grid(gblk, np.zeros_like(gblk), col64, w32, rloc,
                    n_cores, B, nblk, 1)

    # dense edge-expanded features, partition-major bf16
    LT = gA["LT"]
    xeA = np.zeros((n_cores, P, LT * C), dtype=NPBF)
    for cix in range(n_cores):
        fe = x16[gA["ecol"][cix] % NP]
        fe[~gA["used"][cix]] = 0
        xeA[cix] = np.ascontiguousarray(
            fe.reshape(LT, P, C).transpose(1, 0, 2).reshape(P, LT * C))

    # ---- AllGather groups: NG block ranges, each flat size <= 32768 ----
    NG = min(NCHUNK, B)
    nb_g = [B // NG + (1 if g < B % NG else 0) for g in range(NG)]
    blk0_g = np.concatenate([[0], np.cumsum(nb_g)]).astype(np.int64)
    CHW = [n_cores * nb * P for nb in nb_g]      # flat rows per window
    assert all(ch <= 32768 for ch in CHW)
    flat_off = np.concatenate([[0], np.cumsum(CHW)]).astype(np.int64)
    assert flat_off[-1] == NP

    flat_block_off = np.zeros((n_cores, B), dtype=np.int64)
    for g in range(NG):
        b0 = int(blk0_g[g])
        for cix in range(n_cores):
            for b in range(b0, b0 + nb_g[g]):
                flat_block_off[cix, b] = (flat_off[g]
                                          + cix * nb_g[g] * P
                                          + (b - b0) * P)

    # ---- pass B grid: chunk = AllGather window of the source ----
    src_core = col64 // S
    src_b = (col64 % S) // P
    src_r = col64 % P
    flat_src = (flat_block_off[src_core, src_b] + src_r).astype(np.int64)
    gwin = np.searchsorted(flat_off[1:], flat_src, side="right")
    winrel = flat_src - flat_off[gwin]
    gB = _edge_grid(gblk, gwin, winrel, w32, rloc, n_cores, B, nblk, NG)
    LTB = gB["LT"]
    eidxB = gB["ecol"].astype(np.int16)
    eidxB16 = np.ascontiguousarray(
        np.tile(eidxB.reshape(n_cores, (P * LTB) // 16, 16)
                .transpose(0, 2, 1), (1, 8, 1)))

    x_pad16 = np.zeros((NP, C), dtype=NPBF)
    x_pad16[:N] = x16
    xT = np.ascontiguousarray(
        x_pad16.reshape(n_cores, S, C).transpose(0, 2, 1))

    d1_sb = np.ascontiguousarray(d1.reshape(n_cores, S))
    d2_sb = np.ascontiguousarray(d2.reshape(n_cores, S))

    ch_half = -(-NG // 2)          # sweep 0 = windows [0, ch_half)

    return dict(N=N, C=C, S=S, NP=NP, B=B, NG=NG, nb_g=nb_g,
                blk0_g=blk0_g, CHW=CHW, ch_half=ch_half,
                gA=gA, gB=gB, LT=LT, LTB=LTB,
                xeA=xeA, eidxB16=eidxB16, xT=xT, d1=d1_sb, d2=d2_sb)


def _make_stages(T_B, cap=STAGE_TILE_CAP):
    stages = []
    b = 0
    off = 0
    B = len(T_B)
    while b < B:
        start = b
        soff = off
        tot = 0
        while b < B and (b == start or tot + T_B[b] <= cap):
            tot += int(T_B[b])
            off += int(T_B[b])
            b += 1
        stages.append((start, b - start, soff))
    return stages


# ---------------------------------------------------------------------------
# device program
# ---------------------------------------------------------------------------

def build_program(meta, n_cores=N_CORES):
    C, S, NP, B = meta["C"], meta["S"], meta["NP"], meta["B"]
    NG, nb_g, blk0_g, CHW = (meta["NG"], meta["nb_g"], meta["blk0_g"],
                             meta["CHW"])
    ch_half = meta["ch_half"]
    gA, gB = meta["gA"], meta["gB"]
    LT, LTB = meta["LT"], meta["LTB"]
    stages = gA["stages"]
    stagesB = gB["stages"]
    ts_max = max(
        max(int(gA["T_B"][b0:b0 + nb].sum()) for b0, nb, _ in stages),
        max(int(gB["T_B"][b0:b0 + nb].sum()) for b0, nb, _ in stagesB))
    NBMAX = max(max(nb for _, nb, _ in stages),
                max(nb for _, nb, _ in stagesB))

    nc = bacc.Bacc("TRN2", target_bir_lowering=False, debug=False,
                   num_devices=n_cores, num_swdge_queues=4)

    xeA_d = nc.dram_tensor("xeA", [P, LT * C], BF16, kind="ExternalInput")
    emeta_d = nc.dram_tensor("emeta", [P, LT, 4], F32, kind="ExternalInput")
    emetaB_d = nc.dram_tensor("emetaB", [P, LTB, 4], F32,
                              kind="ExternalInput")
    eidxB_d = nc.dram_tensor("eidxB", [P, (P * LTB) // 16], I16,
                             kind="ExternalInput")
    xT_d = nc.dram_tensor("xT", [C, S], BF16, kind="ExternalInput")
    wmat_d = nc.dram_tensor("wmat", [C, 3 * C], BF16, kind="ExternalInput")
    consts_d = nc.dram_tensor("consts", [P, 2 * P], BF16,
                              kind="ExternalInput")
    consts32_d = nc.dram_tensor("consts32", [P, 3 * C + 2 * B], F32,
                                kind="ExternalInput")
    out_d = nc.dram_tensor("out", [S, 3 * C], F32, kind="ExternalOutput")

    sel_ctr = [0]

    with tile.TileContext(nc) as tc:
        with (
            tc.tile_pool(name="cpool", bufs=1) as cpool,
            tc.tile_pool(name="fpool", bufs=3) as fpool,
            tc.tile_pool(name="fpoolB", bufs=3) as fpoolB,
            tc.tile_pool(name="mpool", bufs=3) as mpool,
            tc.tile_pool(name="mpoolB", bufs=3) as mpoolB,
            tc.tile_pool(name="spool", bufs=10) as spool,
            tc.tile_pool(name="zpool", bufs=4) as zpool,
            tc.tile_pool(name="vpool", bufs=2) as vpool,
            tc.tile_pool(name="ypsum", bufs=3, space="PSUM") as ypsum,
            tc.tile_pool(name="tpsum", bufs=2, space="PSUM") as tpsum,
            tc.tile_pool(name="xpsum", bufs=3, space="PSUM") as xpsum,
            tc.tile_pool(name="dram2", bufs=1, space="DRAM") as dram,
        ):
            y1s = dram.tile([S, C], BF16)
            y1fg = []
            for g in range(NG):
                y1fg_g = dram.tile([CHW[g], C], BF16, addr_space="Shared",
                                   name=f"y1fg{g}")
                y1fg.append(y1fg_g)

            consts_t = cpool.tile([P, 2 * P], BF16, tag="consts")
            nc.sync.dma_start(consts_t[:], consts_d[:])
            iota_t = consts_t[:, 0 * P:1 * P]
            eye_t = consts_t[:, 1 * P:2 * P]
            consts32_t = cpool.tile([P, 3 * C + 2 * B], F32, tag="c32")
            nc.sync.dma_start(consts32_t[:], consts32_d[:])
            b0b = consts32_t[:, 0:C]
            b1b = consts32_t[:, C:2 * C]
            b2b = consts32_t[:, 2 * C:3 * C]
            d1col = consts32_t[:, 3 * C:3 * C + B]
            d2col = consts32_t[:, 3 * C + B:3 * C + 2 * B]
            wmat_t = cpool.tile([C, 3 * C], BF16, tag="wmat")
            nc.sync.dma_start(wmat_t[:], wmat_d[:])
            w0_t = wmat_t[:, 0 * C:1 * C]
            w1_t = wmat_t[:, 1 * C:2 * C]
            w2_t = wmat_t[:, 2 * C:3 * C]
            ystg0 = cpool.tile([P, B, C], BF16, tag="ystg0")
            ystg1 = cpool.tile([P, B, C], BF16, tag="ystg1")
            ystg = [ystg0, ystg1]
            nc.vector.memset(ystg[0][:], 0.0)
            nc.vector.memset(ystg[1][:], 0.0)

            def make_sel(row_ap, nrow_ap, w_ap, nw_ap):
                sel_ctr[0] += 1
                sel = spool.tile([P, P], BF16, tag="sel")
                if sel_ctr[0] % SEL_ACT_EVERY == 0:
                    z2 = zpool.tile([P, P], BF16, tag="z2")
                    nc.scalar.activation(
                        out=z2[:], in_=iota_t,
                        func=mybir.ActivationFunctionType.Square,
                        bias=nrow_ap, scale=1.0)
                    nc.scalar.activation(
                        out=sel[:], in_=z2[:],
                        func=mybir.ActivationFunctionType.Relu,
                        bias=w_ap, scale=nw_ap)
                else:
                    nc.vector.tensor_scalar(
                        out=sel[:], in0=iota_t,
                        scalar1=row_ap, scalar2=w_ap,
                        op0=mybir.AluOpType.is_equal,
                        op1=mybir.AluOpType.mult)
                return sel

            # ---------------- pass A (+ inline chunked AllGather) --------
            def emit_ag(g):
                r0 = int(blk0_g[g]) * P
                rows = nb_g[g] * P
                nc.gpsimd.collective_compute(
                    "AllGather",
                    mybir.AluOpType.bypass,
                    replica_groups=[list(range(n_cores))],
                    ins=[y1s[r0:r0 + rows, :].opt()],
                    outs=[y1fg[g][:].opt()],
                )

            # Every AG window fires inline in pass A right after its
            # blocks complete (pass A has no other gpsimd work, so the
            # blocking collective dispatch hides under pass-A compute;
            # only the last window's transfer is exposed).
            n_inline = NG
            ag_after_stage = {}
            for g in range(n_inline):
                lb = int(blk0_g[g + 1]) - 1
                for si, (b0, nb, _) in enumerate(stages):
                    if b0 <= lb < b0 + nb:
                        ag_after_stage.setdefault(si, []).append(g)
                        break

            for si, (b0, nb, soff) in enumerate(stages):
                ts = int(gA["T_B"][b0:b0 + nb].sum())
                mrow = mpool.tile([P, ts_max, 4], F32, tag="meta")
                nc.sync.dma_start(mrow[:, :ts, :],
                                  emeta_d[:, soff:soff + ts, :])
                f_t = fpool.tile([P, ts_max, C], BF16, tag="f")
                nc.sync.dma_start(
                    f_t[:, :ts, :],
                    xeA_d[:, soff * C:(soff + ts) * C]
                    .rearrange("p (t c) -> p t c", c=C))
                y_st = vpool.tile([P, NBMAX, C], BF16, tag="yst")
                for b in range(b0, b0 + nb):
                    kk = b - b0
                    y_ps = ypsum.tile([P, C], F32, tag="ypsum")
                    g0 = int(gA["base_bc"][b, 0] - soff)
                    ntile = int(gA["T_BC"][b, 0])
                    for k in range(ntile):
                        t = g0 + k
                        sel = make_sel(mrow[:, t, 0:1], mrow[:, t, 1:2],
                                       mrow[:, t, 2:3], mrow[:, t, 3:4])
                        nc.tensor.matmul(
                            out=y_ps[:], lhsT=sel[:], rhs=f_t[:, t, :],
                            start=(k == 0), stop=(k == ntile - 1))
                    nc.vector.tensor_copy(y_st[:, kk, :], y_ps[:])
                nc.sync.dma_start(
                    y1s[b0 * P:(b0 + nb) * P, :]
                    .rearrange("(g p) c -> p g c", p=P),
                    y_st[:, :nb, :])
                for g in ag_after_stage.get(si, []):
                    emit_ag(g)

            # ---------------- dense x0/x1 loops (overlap AG) -------------
            def emit_x0_x1(gsz=4):
                for g0 in range(0, B, gsz):
                    gn = min(gsz, B - g0)
                    xT_t = vpool.tile([C, gsz * P], BF16, tag="xT")
                    nc.sync.dma_start(xT_t[:, :gn * P],
                                      xT_d[:, g0 * P:(g0 + gn) * P])
                    y_ld = vpool.tile([P, gsz, C], BF16, tag="yld")
                    nc.sync.dma_start(
                        y_ld[:, :gn, :],
                        y1s[g0 * P:(g0 + gn) * P, :]
                        .rearrange("(g p) c -> p g c", p=P))
                    x01_st = vpool.tile([P, gsz, 2 * C], F32, tag="x01st")
                    for k in range(gn):
                        b = g0 + k
                        x0_ps = xpsum.tile([P, C], F32, tag="xpsum")
                        nc.tensor.matmul(
                            out=x0_ps[:], lhsT=xT_t[:, k * P:(k + 1) * P],
                            rhs=w0_t, start=True, stop=True)
                        nc.vector.tensor_tensor(
                            out=x01_st[:, k, 0:C], in0=x0_ps[:], in1=b0b,
                            op=mybir.AluOpType.add)
                        yT_ps = tpsum.tile([P, C], BF16, tag="tpsum")
                        nc.tensor.transpose(yT_ps[:], y_ld[:, k, :], eye_t)
                        yT_sb = vpool.tile([P, C], BF16, tag="ytsb")
                        nc.scalar.activation(
                            out=yT_sb[:], in_=yT_ps[:],
                            func=mybir.ActivationFunctionType.Copy,
                            scale=1.0)
                        x1_ps = xpsum.tile([P, C], F32, tag="xpsum")
                        nc.tensor.matmul(out=x1_ps[:], lhsT=yT_sb[:],
                                         rhs=w1_t, start=True, stop=True)
                        tmp1 = zpool.tile([P, C], F32, tag="tmp")
                        nc.scalar.activation(
                            out=tmp1[:], in_=b1b,
                            func=mybir.ActivationFunctionType.Copy,
                            scale=d1col[:, b:b + 1])
                        nc.vector.tensor_tensor(
                            out=x01_st[:, k, C:2 * C], in0=x1_ps[:],
                            in1=tmp1[:], op=mybir.AluOpType.add)
                    nc.sync.dma_start(
                        out_d[g0 * P:(g0 + gn) * P, 0:2 * C]
                        .rearrange("(g p) c -> p g c", p=P),
                        x01_st[:, :gn, :])

            # ---------------- pass B: one sweep per source window ---------
            def emit_x2_blocks(b0, nb):
                x2_st = vpool.tile([P, NBMAX, C], F32, tag="x2st")
                for b in range(b0, b0 + nb):
                    kk = b - b0
                    x_ps = xpsum.tile([P, C], F32, tag="xpsum")
                    nc.tensor.matmul(out=x_ps[:], lhsT=ystg[0][:, b, :],
                                     rhs=w2_t, start=True, stop=False)
                    nc.tensor.matmul(out=x_ps[:], lhsT=ystg[1][:, b, :],
                                     rhs=w2_t, start=False, stop=True)
                    tmp2 = zpool.tile([P, C], F32, tag="tmp")
                    nc.scalar.activation(
                        out=tmp2[:], in_=b2b,
                        func=mybir.ActivationFunctionType.Copy,
                        scale=d2col[:, b:b + 1])
                    nc.vector.tensor_tensor(
                        out=x2_st[:, kk, :], in0=x_ps[:],
                        in1=tmp2[:], op=mybir.AluOpType.add)
                nc.sync.dma_start(
                    out_d[b0 * P:(b0 + nb) * P, 2 * C:3 * C]
                    .rearrange("(g p) c -> p g c", p=P),
                    x2_st[:, :nb, :])

            def emit_passB_sweep(sw):
                clo = 0 if sw == 0 else ch_half
                chi = ch_half if sw == 0 else NG
                last = (sw == 1)
                for si, (b0, nb, soff) in enumerate(stagesB):
                    lens = gB["call_lens"][si]
                    rel0 = sum(lens[:clo])
                    ts_sw = sum(lens[clo:chi])
                    if ts_sw == 0:
                        if last:
                            emit_x2_blocks(b0, nb)
                        continue
                    so = soff + rel0
                    mrow = mpoolB.tile([P, ts_max, 4], F32, tag="metaB")
                    nc.sync.dma_start(mrow[:, :ts_sw, :],
                                      emetaB_d[:, so:so + ts_sw, :])
                    idx_t = mpoolB.tile([P, (P * ts_max) // 16], I16,
                                        tag="idxB")
                    i16o = (P * so) // 16
                    i16n = (P * ts_sw) // 16
                    nc.sync.dma_start(idx_t[:, :i16n],
                                      eidxB_d[:, i16o:i16o + i16n])
                    f_t = fpoolB.tile([P, ts_max, C], BF16, tag="fB")
                    rel = 0
                    for c in range(clo, chi):
                        tsc = lens[c]
                        if tsc == 0:
                            continue
                        nidx = P * tsc
                        nc.gpsimd.dma_gather(
                            out_ap=f_t[:, rel:rel + tsc, :],
                            in_ap=y1fg[c][:, :],
                            idxs_ap=idx_t[:, (P * rel) // 16:
                                          (P * rel) // 16 + nidx // 16],
                            num_idxs=nidx,
                            num_idxs_reg=nidx,
                            elem_size=C,
                            single_packet=False,
                            queue_num=(si + c) % 4,
                        )
                        rel += tsc
                    for b in range(b0, b0 + nb):
                        tl = [(int(gB["base_bc"][b, c] - so),
                               int(gB["T_BC"][b, c]))
                              for c in range(clo, chi)
                              if gB["T_BC"][b, c] > 0]
                        ntile = sum(n for _, n in tl)
                        if ntile == 0:
                            continue
                        y_ps = ypsum.tile([P, C], F32, tag="ypsum")
                        k = 0
                        for (g0, n) in tl:
                            for t in range(g0, g0 + n):
                                sel = make_sel(mrow[:, t, 0:1],
                                               mrow[:, t, 1:2],
                                               mrow[:, t, 2:3],
                                               mrow[:, t, 3:4])
                                nc.tensor.matmul(
                                    out=y_ps[:], lhsT=f_t[:, t, :],
                                    rhs=sel[:],
                                    start=(k == 0), stop=(k == ntile - 1))
                                k += 1
                        nc.scalar.activation(
                            out=ystg[sw][:, b, :], in_=y_ps[:],
                            func=mybir.ActivationFunctionType.Copy,
                            scale=1.0)
                    if last:
                        emit_x2_blocks(b0, nb)

            emit_passB_sweep(0)
            emit_x0_x1()
            emit_passB_sweep(1)

    nc.compile()
    return nc


# ---------------------------------------------------------------------------
# entry point
# ---------------------------------------------------------------------------

def make_in_maps(meta, W0, b0, W1, b1, W2, b2, n_cores=N_CORES):
    C, S = meta["C"], meta["S"]
    iota = np.tile(np.arange(P, dtype=np.float32), (P, 1)).astype(NPBF)
    eye = np.eye(P, dtype=np.float32).astype(NPBF)
    consts = np.ascontiguousarray(np.concatenate([iota, eye], axis=1))
    wmat = np.concatenate(
        [np.asarray(W0, np.float32), np.asarray(W1, np.float32),
         np.asarray(W2, np.float32)], axis=1).astype(NPBF)
    B = meta["B"]
    b0b = np.tile(np.asarray(b0, np.float32), (P, 1))
    b1b = np.tile(np.asarray(b1, np.float32), (P, 1))
    b2b = np.tile(np.asarray(b2, np.float32), (P, 1))
    in_maps = []
    for c in range(n_cores):
        d1c = np.ascontiguousarray(
            meta["d1"][c].reshape(B, P).T)          # [128, B]
        d2c = np.ascontiguousarray(
            meta["d2"][c].reshape(B, P).T)
        consts32 = np.ascontiguousarray(np.concatenate(
            [b0b, b1b, b2b, d1c, d2c], axis=1))
        in_maps.append({
            "xeA": meta["xeA"][c],
            "emeta": _meta_tile(meta["gA"]["emeta"][c], meta["LT"]),
            "emetaB": _meta_tile(meta["gB"]["emeta"][c], meta["LTB"]),
            "eidxB": meta["eidxB16"][c],
            "xT": meta["xT"][c],
            "wmat": wmat,
            "consts": consts,
            "consts32": consts32,
        })
    return in_maps


def _meta_tile(em, LT_):
    # [L, 4] edge-order -> [128, LT, 4]: edge j -> partition j%128, tile j//128
    return np.ascontiguousarray(em.reshape(LT_, P, 4).transpose(1, 0, 2))


def kernel(x, row, col, edge_weight, W0, b0, W1, b1, W2, b2):
    x = np.asarray(x, np.float32)
    row = np.asarray(row, np.int32)
    col = np.asarray(col, np.int32)
    edge_weight = np.asarray(edge_weight, np.float32)
    N = x.shape[0]

    meta = _prep(x, row, col, edge_weight)
    nc = build_program(meta)
    in_maps = make_in_maps(meta, W0, b0, W1, b1, W2, b2)
    res = run_bass_kernel_spmd(nc, in_maps, core_ids=list(range(N_CORES)))
    out = np.concatenate([r["out"] for r in res.results], axis=0)
    return np.ascontiguousarray(out[:N])


if __name__ == "__main__":
    rng = np.random.default_rng(0)
    N, C, E = 2048, 128, 8192
    x = rng.standard_normal((N, C), dtype=np.float32)
    row = rng.integers(0, N, E).astype(np.int32)
    col = rng.integers(0, N, E).astype(np.int32)
    w = rng.random(E, dtype=np.float32)
    meta = _prep(x, row, col, w)
    print("tiles A:", meta["LT"], "tiles B:", meta["LTB"],
          "stages:", len(meta["gA"]["stages"]), len(meta["gB"]["stages"]))


# revision 15
# speedup vs baseline: 1.4326x; 1.0017x over previous
"""MixHop layer (gnn_message_passing) as a Trainium2 Bass/Tile SPMD kernel.

Math reformulation (A = sparse adjacency with edge weights, row=dst, col=src):
    x0 = x @ W0 + b0
    x1 = A @ (x @ W1 + b1) = (A @ x) @ W1 + d1 (x) b1      d1 = A @ 1
    x2 = A @ A @ (x @ W2 + b2) = (A @ A @ x) @ W2 + d2 (x) b2,  d2 = A @ d1
so only two sparse propagations of the raw features are needed:
    y1 = A @ x   (pass A),   y2 = A @ y1  (pass B, after all-gather of y1)

v3 design (driven by HW microbenchmarks):
  * dma_gather is descriptor-rate-bound (~5.3ns/desc shared wall), so pass A
    avoids gathers entirely: the host pre-expands x[col[e]] into dense bf16
    streams laid out partition-major; the device streams them with plain
    contiguous DMA.
  * pass B gathers y1 rows (256B bf16) from the all-gathered y1f.
  * All scatter matmuls run in bf16 (fp32 PSUM accumulate).  Selection
    matrices sel[e,r] = w[e]*(row[e]==r) are built per 128-edge tile on
    the Vector engine (tensor_scalar is_equal*mult) with a fraction
    offloaded to the idle Scalar engine as relu(-w*(iota-row)^2 + w).
  * Biases (1xb0, d1xb1, d2xb2) are K=1 rank-1 matmuls appended to the
    dense PSUM groups (PE has headroom; frees the Vector engine).
  * The AllGather of y1 (bf16) is chunked per stage-group and issued
    inline with pass A (pass A has no gpsimd work, so no head-of-line
    blocking); y1f is laid out stage-group-major so pass B's first
    source window is complete early.
  * pass B runs as two half-sweeps by source window: sweep 0 (first half
    of y1f) starts gathering while pass A is still finishing; per-block
    partial results are staged in SBUF and combined by per-block W2
    matmul groups at the end.
"""

import os
import sys

import numpy as np

for _p in ("/opt/trn_rl_repo",):
    if os.path.isdir(_p) and _p not in sys.path:
        sys.path.insert(0, _p)

import ml_dtypes

import concourse.bacc as bacc
import concourse.mybir as mybir
import concourse.tile as tile
from concourse.bass_utils import run_bass_kernel_spmd

F32 = mybir.dt.float32
BF16 = mybir.dt.bfloat16
I16 = mybir.dt.int16
NPBF = ml_dtypes.bfloat16

N_CORES = 8
P = 128          # partitions / rows per block / edges per tile
NCHUNK = 4       # source chunks (int16 index reach)
STAGE_TILE_CAP = 80   # max edge tiles staged in SBUF at once
SEL_ACT_EVERY = 4     # every k-th sel tile goes to the Scalar engine
AG_STAGES = 3         # stages per AllGather chunk


# ---------------------------------------------------------------------------
# host-side preprocessing
# ---------------------------------------------------------------------------

def _edge_grid(gblk, chunk, colrel, w, rloc, n_cores, B, nblk, nchunk):
    """Sort edges by (dest block, chunk); build the padded tile grid.

    Returns per-core tile grid metadata + per-edge slot assignment."""
    order = np.lexsort((chunk, gblk))
    gblk_s = gblk[order]
    chunk_s = chunk[order]
    col_s = colrel[order]
    w_s = w[order]
    rloc_s = rloc[order]

    E = len(gblk)
    grp = gblk_s * nchunk + chunk_s
    cnt = np.bincount(grp, minlength=nblk * nchunk)
    tiles = -(-cnt // P)
    T_BC = tiles.reshape(n_cores, B, nchunk).max(axis=0).astype(np.int64)
    empty = T_BC.sum(axis=1) == 0
    T_BC[empty, 0] = 1
    T_B = T_BC.sum(axis=1)
    LT = int(T_B.sum())

    stages = _make_stages(T_B)

    base_bc = np.zeros((B, nchunk), dtype=np.int64)
    pos0 = 0
    call_lens = []
    for (b0, nb, _) in stages:
        lens = []
        for c in range(nchunk):
            n = 0
            for b in range(b0, b0 + nb):
                base_bc[b, c] = pos0
                pos0 += T_BC[b, c]
                n += T_BC[b, c]
            lens.append(int(n))
        call_lens.append(lens)
    assert pos0 == LT

    grp_start = np.zeros(nblk * nchunk + 1, dtype=np.int64)
    np.cumsum(cnt, out=grp_start[1:])
    rank = np.arange(E, dtype=np.int64) - grp_start[grp]
    b_local = gblk_s % B
    e_core = gblk_s // B
    pos = P * base_bc[b_local, chunk_s] + rank

    L = P * LT
    emeta = np.zeros((n_cores, L, 4), dtype=np.float32)
    emeta[e_core, pos, 0] = rloc_s
    emeta[e_core, pos, 1] = -rloc_s
    emeta[e_core, pos, 2] = w_s
    emeta[e_core, pos, 3] = -w_s
    ecol = np.zeros((n_cores, L), dtype=np.int64)
    ecol[e_core, pos] = col_s
    used = np.zeros((n_cores, L), dtype=bool)
    used[e_core, pos] = True
    return dict(T_BC=T_BC, T_B=T_B, LT=LT, stages=stages,
                call_lens=call_lens, base_bc=base_bc, emeta=emeta,
                ecol=ecol, used=used)


def _prep(x, row, col, edge_weight, n_cores=N_CORES):
    N, C = x.shape
    S = -(-N // (n_cores * P)) * P          # shard rows per core
    NP = S * n_cores                        # padded node count
    B = S // P                              # dest blocks per core
    nblk = NP // P

    w64 = edge_weight.astype(np.float64)
    d1 = np.bincount(row, weights=w64, minlength=NP)[:NP]
    d2 = np.bincount(row, weights=w64 * d1[col], minlength=NP)[:NP]
    d1 = d1.astype(np.float32)
    d2 = d2.astype(np.float32)

    x16 = np.ascontiguousarray(x.astype(NPBF))

    gblk = (row // P).astype(np.int64)
    rloc = (row % P).astype(np.float32)
    w32 = edge_weight.astype(np.float32)
    col64 = col.astype(np.int64)

    # ---- pass A grid: no gathers -> no chunking at all ----
    gA = _edge_grid(gblk, np.zeros_like(gblk), col64, w32, rloc,
                    n_cores, B, nblk, 1)

    # dense edge-expanded features, partition-major bf16
    LT = gA["LT"]
    xeA = np.zeros((n_cores, P, LT * C), dtype=NPBF)
    for cix in range(n_cores):
        fe = x16[gA["ecol"][cix] % NP]
        fe[~gA["used"][cix]] = 0
        xeA[cix] = np.ascontiguousarray(
            fe.reshape(LT, P, C).transpose(1, 0, 2).reshape(P, LT * C))

    # ---- AllGather groups: NG block ranges, each flat size <= 32768 ----
    NG = min(NCHUNK, B)
    nb_g = [B // NG + (1 if g < B % NG else 0) for g in range(NG)]
    blk0_g = np.concatenate([[0], np.cumsum(nb_g)]).astype(np.int64)
    CHW = [n_cores * nb * P for nb in nb_g]      # flat rows per window
    assert all(ch <= 32768 for ch in CHW)
    flat_off = np.concatenate([[0], np.cumsum(CHW)]).astype(np.int64)
    assert flat_off[-1] == NP

    flat_block_off = np.zeros((n_cores, B), dtype=np.int64)
    for g in range(NG):
        b0 = int(blk0_g[g])
        for cix in range(n_cores):
            for b in range(b0, b0 + nb_g[g]):
                flat_block_off[cix, b] = (flat_off[g]
                                          + cix * nb_g[g] * P
                                          + (b - b0) * P)

    # ---- pass B grid: chunk = AllGather window of the source ----
    src_core = col64 // S
    src_b = (col64 % S) // P
    src_r = col64 % P
    flat_src = (flat_block_off[src_core, src_b] + src_r).astype(np.int64)
    gwin = np.searchsorted(flat_off[1:], flat_src, side="right")
    winrel = flat_src - flat_off[gwin]
    gB = _edge_grid(gblk, gwin, winrel, w32, rloc, n_cores, B, nblk, NG)
    LTB = gB["LT"]
    eidxB = gB["ecol"].astype(np.int16)
    eidxB16 = np.ascontiguousarray(
        np.tile(eidxB.reshape(n_cores, (P * LTB) // 16, 16)
                .transpose(0, 2, 1), (1, 8, 1)))

    x_pad16 = np.zeros((NP, C), dtype=NPBF)
    x_pad16[:N] = x16
    xT = np.ascontiguousarray(
        x_pad16.reshape(n_cores, S, C).transpose(0, 2, 1))

    d1_sb = np.ascontiguousarray(d1.reshape(n_cores, S))
    d2_sb = np.ascontiguousarray(d2.reshape(n_cores, S))

    ch_half = -(-NG // 2)          # sweep 0 = windows [0, ch_half)

    return dict(N=N, C=C, S=S, NP=NP, B=B, NG=NG, nb_g=nb_g,
                blk0_g=blk0_g, CHW=CHW, ch_half=ch_half,
                gA=gA, gB=gB, LT=LT, LTB=LTB,
                xeA=xeA, eidxB16=eidxB16, xT=xT, d1=d1_sb, d2=d2_sb)


def _make_stages(T_B, cap=STAGE_TILE_CAP):
    stages = []
    b = 0
    off = 0
    B = len(T_B)
    while b < B:
        start = b
        soff = off
        tot = 0
        while b < B and (b == start or tot + T_B[b] <= cap):
            tot += int(T_B[b])
            off += int(T_B[b])
            b += 1
        stages.append((start, b - start, soff))
    return stages


# ---------------------------------------------------------------------------
# device program
# ---------------------------------------------------------------------------

def build_program(meta, n_cores=N_CORES):
    C, S, NP, B = meta["C"], meta["S"], meta["NP"], meta["B"]
    NG, nb_g, blk0_g, CHW = (meta["NG"], meta["nb_g"], meta["blk0_g"],
                             meta["CHW"])
    ch_half = meta["ch_half"]
    gA, gB = meta["gA"], meta["gB"]
    LT, LTB = meta["LT"], meta["LTB"]
    stages = gA["stages"]
    stagesB = gB["stages"]
    ts_max = max(
        max(int(gA["T_B"][b0:b0 + nb].sum()) for b0, nb, _ in stages),
        max(int(gB["T_B"][b0:b0 + nb].sum()) for b0, nb, _ in stagesB))
    NBMAX = max(max(nb for _, nb, _ in stages),
                max(nb for _, nb, _ in stagesB))

    nc = bacc.Bacc("TRN2", target_bir_lowering=False, debug=False,
                   num_devices=n_cores, num_swdge_queues=4)

    xeA_d = nc.dram_tensor("xeA", [P, LT * C], BF16, kind="ExternalInput")
    emeta_d = nc.dram_tensor("emeta", [P, LT, 4], F32, kind="ExternalInput")
    emetaB_d = nc.dram_tensor("emetaB", [P, LTB, 4], F32,
                              kind="ExternalInput")
    eidxB_d = nc.dram_tensor("eidxB", [P, (P * LTB) // 16], I16,
                             kind="ExternalInput")
    xT_d = nc.dram_tensor("xT", [C, S], BF16, kind="ExternalInput")
    wmat_d = nc.dram_tensor("wmat", [C, 3 * C], BF16, kind="ExternalInput")
    consts_d = nc.dram_tensor("consts", [P, 2 * P], BF16,
                              kind="ExternalInput")
    consts32_d = nc.dram_tensor("consts32", [P, 3 * C + 2 * B], F32,
                                kind="ExternalInput")
    out_d = nc.dram_tensor("out", [S, 3 * C], F32, kind="ExternalOutput")

    sel_ctr = [0]

    with tile.TileContext(nc) as tc:
        with (
            tc.tile_pool(name="cpool", bufs=1) as cpool,
            tc.tile_pool(name="fpool", bufs=3) as fpool,
            tc.tile_pool(name="fpoolB", bufs=3) as fpoolB,
            tc.tile_pool(name="mpool", bufs=3) as mpool,
            tc.tile_pool(name="mpoolB", bufs=3) as mpoolB,
            tc.tile_pool(name="spool", bufs=10) as spool,
            tc.tile_pool(name="zpool", bufs=4) as zpool,
            tc.tile_pool(name="vpool", bufs=2) as vpool,
            tc.tile_pool(name="ypsum", bufs=3, space="PSUM") as ypsum,
            tc.tile_pool(name="tpsum", bufs=2, space="PSUM") as tpsum,
            tc.tile_pool(name="xpsum", bufs=3, space="PSUM") as xpsum,
            tc.tile_pool(name="dram2", bufs=1, space="DRAM") as dram,
        ):
            y1s = dram.tile([S, C], BF16)
            y1fg = []
            for g in range(NG):
                y1fg_g = dram.tile([CHW[g], C], BF16, addr_space="Shared",
                                   name=f"y1fg{g}")
                y1fg.append(y1fg_g)

            consts_t = cpool.tile([P, 2 * P], BF16, tag="consts")
            nc.sync.dma_start(consts_t[:], consts_d[:])
            iota_t = consts_t[:, 0 * P:1 * P]
            eye_t = consts_t[:, 1 * P:2 * P]
            consts32_t = cpool.tile([P, 3 * C + 2 * B], F32, tag="c32")
            nc.sync.dma_start(consts32_t[:], consts32_d[:])
            b0b = consts32_t[:, 0:C]
            b1b = consts32_t[:, C:2 * C]
            b2b = consts32_t[:, 2 * C:3 * C]
            d1col = consts32_t[:, 3 * C:3 * C + B]
            d2col = consts32_t[:, 3 * C + B:3 * C + 2 * B]
            wmat_t = cpool.tile([C, 3 * C], BF16, tag="wmat")
            nc.sync.dma_start(wmat_t[:], wmat_d[:])
            w0_t = wmat_t[:, 0 * C:1 * C]
            w1_t = wmat_t[:, 1 * C:2 * C]
            w2_t = wmat_t[:, 2 * C:3 * C]
            ystg0 = cpool.tile([P, B, C], BF16, tag="ystg0")
            ystg1 = cpool.tile([P, B, C], BF16, tag="ystg1")
            ystg = [ystg0, ystg1]
            nc.vector.memset(ystg[0][:], 0.0)
            nc.vector.memset(ystg[1][:], 0.0)

            def make_sel(row_ap, nrow_ap, w_ap, nw_ap):
                sel_ctr[0] += 1
                sel = spool.tile([P, P], BF16, tag="sel")
                if sel_ctr[0] % SEL_ACT_EVERY == 0:
                    z2 = zpool.tile([P, P], BF16, tag="z2")
                    nc.scalar.activation(
                        out=z2[:], in_=iota_t,
                        func=mybir.ActivationFunctionType.Square,
                        bias=nrow_ap, scale=1.0)
                    nc.scalar.activation(
                        out=sel[:], in_=z2[:],
                        func=mybir.ActivationFunctionType.Relu,
                        bias=w_ap, scale=nw_ap)
                else:
                    nc.vector.tensor_scalar(
                        out=sel[:], in0=iota_t,
                        scalar1=row_ap, scalar2=w_ap,
                        op0=mybir.AluOpType.is_equal,
                        op1=mybir.AluOpType.mult)
                return sel

            # ---------------- pass A (+ inline chunked AllGather) --------
            def emit_ag(g):
                r0 = int(blk0_g[g]) * P
                rows = nb_g[g] * P
                nc.gpsimd.collective_compute(
                    "AllGather",
                    mybir.AluOpType.bypass,
                    replica_groups=[list(range(n_cores))],
                    ins=[y1s[r0:r0 + rows, :].opt()],
                    outs=[y1fg[g][:].opt()],
                )

            # AG windows 0..NG-2 fire inline in pass A right after their
            # blocks complete (pass A has no other gpsimd work, so the
            # blocking collective dispatch hides under pass-A compute).
            # The last window is dispatched a few stages into sweep 0:
            # its y1s rows are complete by then, and the already-issued
            # sweep-0 gather calls keep draining under its transfer.
            n_inline = max(1, NG - 1)
            ag_after_stage = {}
            for g in range(n_inline):
                lb = int(blk0_g[g + 1]) - 1
                for si, (b0, nb, _) in enumerate(stages):
                    if b0 <= lb < b0 + nb:
                        ag_after_stage.setdefault(si, []).append(g)
                        break

            for si, (b0, nb, soff) in enumerate(stages):
                ts = int(gA["T_B"][b0:b0 + nb].sum())
                mrow = mpool.tile([P, ts_max, 4], F32, tag="meta")
                nc.sync.dma_start(mrow[:, :ts, :],
                                  emeta_d[:, soff:soff + ts, :])
                f_t = fpool.tile([P, ts_max, C], BF16, tag="f")
                nc.sync.dma_start(
                    f_t[:, :ts, :],
                    xeA_d[:, soff * C:(soff + ts) * C]
                    .rearrange("p (t c) -> p t c", c=C))
                y_st = vpool.tile([P, NBMAX, C], BF16, tag="yst")
                for b in range(b0, b0 + nb):
                    kk = b - b0
                    y_ps = ypsum.tile([P, C], F32, tag="ypsum")
                    g0 = int(gA["base_bc"][b, 0] - soff)
                    ntile = int(gA["T_BC"][b, 0])
                    for k in range(ntile):
                        t = g0 + k
                        sel = make_sel(mrow[:, t, 0:1], mrow[:, t, 1:2],
                                       mrow[:, t, 2:3], mrow[:, t, 3:4])
                        nc.tensor.matmul(
                            out=y_ps[:], lhsT=sel[:], rhs=f_t[:, t, :],
                            start=(k == 0), stop=(k == ntile - 1))
                    nc.vector.tensor_copy(y_st[:, kk, :], y_ps[:])
                nc.sync.dma_start(
                    y1s[b0 * P:(b0 + nb) * P, :]
                    .rearrange("(g p) c -> p g c", p=P),
                    y_st[:, :nb, :])
                for g in ag_after_stage.get(si, []):
                    emit_ag(g)

            # ---------------- dense x0/x1 loops (overlap AG) -------------
            def emit_x0_x1(gsz=4):
                for g0 in range(0, B, gsz):
                    gn = min(gsz, B - g0)
                    xT_t = vpool.tile([C, gsz * P], BF16, tag="xT")
                    nc.sync.dma_start(xT_t[:, :gn * P],
                                      xT_d[:, g0 * P:(g0 + gn) * P])
                    y_ld = vpool.tile([P, gsz, C], BF16, tag="yld")
                    nc.sync.dma_start(
                        y_ld[:, :gn, :],
                        y1s[g0 * P:(g0 + gn) * P, :]
                        .rearrange("(g p) c -> p g c", p=P))
                    x01_st = vpool.tile([P, gsz, 2 * C], F32, tag="x01st")
                    for k in range(gn):
                        b = g0 + k
                        x0_ps = xpsum.tile([P, C], F32, tag="xpsum")
                        nc.tensor.matmul(
                            out=x0_ps[:], lhsT=xT_t[:, k * P:(k + 1) * P],
                            rhs=w0_t, start=True, stop=True)
                        nc.vector.tensor_tensor(
                            out=x01_st[:, k, 0:C], in0=x0_ps[:], in1=b0b,
                            op=mybir.AluOpType.add)
                        yT_ps = tpsum.tile([P, C], BF16, tag="tpsum")
                        nc.tensor.transpose(yT_ps[:], y_ld[:, k, :], eye_t)
                        yT_sb = vpool.tile([P, C], BF16, tag="ytsb")
                        nc.scalar.activation(
                            out=yT_sb[:], in_=yT_ps[:],
                            func=mybir.ActivationFunctionType.Copy,
                            scale=1.0)
                        x1_ps = xpsum.tile([P, C], F32, tag="xpsum")
                        nc.tensor.matmul(out=x1_ps[:], lhsT=yT_sb[:],
                                         rhs=w1_t, start=True, stop=True)
                        tmp1 = zpool.tile([P, C], F32, tag="tmp")
                        nc.scalar.activation(
                            out=tmp1[:], in_=b1b,
                            func=mybir.ActivationFunctionType.Copy,
                            scale=d1col[:, b:b + 1])
                        nc.vector.tensor_tensor(
                            out=x01_st[:, k, C:2 * C], in0=x1_ps[:],
                            in1=tmp1[:], op=mybir.AluOpType.add)
                    nc.sync.dma_start(
                        out_d[g0 * P:(g0 + gn) * P, 0:2 * C]
                        .rearrange("(g p) c -> p g c", p=P),
                        x01_st[:, :gn, :])

            # ---------------- pass B: one sweep per source window ---------
            def emit_x2_blocks(b0, nb):
                x2_st = vpool.tile([P, NBMAX, C], F32, tag="x2st")
                for b in range(b0, b0 + nb):
                    kk = b - b0
                    x_ps = xpsum.tile([P, C], F32, tag="xpsum")
                    nc.tensor.matmul(out=x_ps[:], lhsT=ystg[0][:, b, :],
                                     rhs=w2_t, start=True, stop=False)
                    nc.tensor.matmul(out=x_ps[:], lhsT=ystg[1][:, b, :],
                                     rhs=w2_t, start=False, stop=True)
                    tmp2 = zpool.tile([P, C], F32, tag="tmp")
                    nc.scalar.activation(
                        out=tmp2[:], in_=b2b,
                        func=mybir.ActivationFunctionType.Copy,
                        scale=d2col[:, b:b + 1])
                    nc.vector.tensor_tensor(
                        out=x2_st[:, kk, :], in0=x_ps[:],
                        in1=tmp2[:], op=mybir.AluOpType.add)
                nc.sync.dma_start(
                    out_d[b0 * P:(b0 + nb) * P, 2 * C:3 * C]
                    .rearrange("(g p) c -> p g c", p=P),
                    x2_st[:, :nb, :])

            def emit_passB_sweep(sw):
                clo = 0 if sw == 0 else ch_half
                chi = ch_half if sw == 0 else NG
                last = (sw == 1)
                for si, (b0, nb, soff) in enumerate(stagesB):
                    if sw == 0 and si == min(3, len(stagesB) - 1):
                        for g in range(n_inline, NG):
                            emit_ag(g)
                    lens = gB["call_lens"][si]
                    rel0 = sum(lens[:clo])
                    ts_sw = sum(lens[clo:chi])
                    if ts_sw == 0:
                        if last:
                            emit_x2_blocks(b0, nb)
                        continue
                    so = soff + rel0
                    mrow = mpoolB.tile([P, ts_max, 4], F32, tag="metaB")
                    nc.sync.dma_start(mrow[:, :ts_sw, :],
                                      emetaB_d[:, so:so + ts_sw, :])
                    idx_t = mpoolB.tile([P, (P * ts_max) // 16], I16,
                                        tag="idxB")
                    i16o = (P * so) // 16
                    i16n = (P * ts_sw) // 16
                    nc.sync.dma_start(idx_t[:, :i16n],
                                      eidxB_d[:, i16o:i16o + i16n])
                    f_t = fpoolB.tile([P, ts_max, C], BF16, tag="fB")
                    rel = 0
                    for c in range(clo, chi):
                        tsc = lens[c]
                        if tsc == 0:
                            continue
                        nidx = P * tsc
                        nc.gpsimd.dma_gather(
                            out_ap=f_t[:, rel:rel + tsc, :],
                            in_ap=y1fg[c][:, :],
                            idxs_ap=idx_t[:, (P * rel) // 16:
                                          (P * rel) // 16 + nidx // 16],
                            num_idxs=nidx,
                            num_idxs_reg=nidx,
                            elem_size=C,
                            single_packet=False,
                            queue_num=(si + c) % 4,
                        )
                        rel += tsc
                    for b in range(b0, b0 + nb):
                        tl = [(int(gB["base_bc"][b, c] - so),
                               int(gB["T_BC"][b, c]))
                              for c in range(clo, chi)
                              if gB["T_BC"][b, c] > 0]
                        ntile = sum(n for _, n in tl)
                        if ntile == 0:
                            continue
                        y_ps = ypsum.tile([P, C], F32, tag="ypsum")
                        k = 0
                        for (g0, n) in tl:
                            for t in range(g0, g0 + n):
                                sel = make_sel(mrow[:, t, 0:1],
                                               mrow[:, t, 1:2],
                                               mrow[:, t, 2:3],
                                               mrow[:, t, 3:4])
                                nc.tensor.matmul(
                                    out=y_ps[:], lhsT=f_t[:, t, :],
                                    rhs=sel[:],
                                    start=(k == 0), stop=(k == ntile - 1))
                                k += 1
                        nc.scalar.activation(
                            out=ystg[sw][:, b, :], in_=y_ps[:],
                            func=mybir.ActivationFunctionType.Copy,
                            scale=1.0)
                    if last:
                        emit_x2_blocks(b0, nb)

            emit_x0_x1()
            emit_passB_sweep(0)
            emit_passB_sweep(1)

    nc.compile()
    return nc


# ---------------------------------------------------------------------------
# entry point
# ---------------------------------------------------------------------------

def make_in_maps(meta, W0, b0, W1, b1, W2, b2, n_cores=N_CORES):
    C, S = meta["C"], meta["S"]
    iota = np.tile(np.arange(P, dtype=np.float32), (P, 1)).astype(NPBF)
    eye = np.eye(P, dtype=np.float32).astype(NPBF)
    consts = np.ascontiguousarray(np.concatenate([iota, eye], axis=1))
    wmat = np.concatenate(
        [np.asarray(W0, np.float32), np.asarray(W1, np.float32),
         np.asarray(W2, np.float32)], axis=1).astype(NPBF)
    B = meta["B"]
    b0b = np.tile(np.asarray(b0, np.float32), (P, 1))
    b1b = np.tile(np.asarray(b1, np.float32), (P, 1))
    b2b = np.tile(np.asarray(b2, np.float32), (P, 1))
    in_maps = []
    for c in range(n_cores):
        d1c = np.ascontiguousarray(
            meta["d1"][c].reshape(B, P).T)          # [128, B]
        d2c = np.ascontiguousarray(
            meta["d2"][c].reshape(B, P).T)
        consts32 = np.ascontiguousarray(np.concatenate(
            [b0b, b1b, b2b, d1c, d2c], axis=1))
        in_maps.append({
            "xeA": meta["xeA"][c],
            "emeta": _meta_tile(meta["gA"]["emeta"][c], meta["LT"]),
            "emetaB": _meta_tile(meta["gB"]["emeta"][c], meta["LTB"]),
            "eidxB": meta["eidxB16"][c],
            "xT": meta["xT"][c],
            "wmat": wmat,
            "consts": consts,
            "consts32": consts32,
        })
    return in_maps


def _meta_tile(em, LT_):
    # [L, 4] edge-order -> [128, LT, 4]: edge j -> partition j%128, tile j//128
    return np.ascontiguousarray(em.reshape(LT_, P, 4).transpose(1, 0, 2))


def kernel(x, row, col, edge_weight, W0, b0, W1, b1, W2, b2):
    x = np.asarray(x, np.float32)
    row = np.asarray(row, np.int32)
    col = np.asarray(col, np.int32)
    edge_weight = np.asarray(edge_weight, np.float32)
    N = x.shape[0]

    meta = _prep(x, row, col, edge_weight)
    nc = build_program(meta)
    in_maps = make_in_maps(meta, W0, b0, W1, b1, W2, b2)
    res = run_bass_kernel_spmd(nc, in_maps, core_ids=list(range(N_CORES)))
    out = np.concatenate([r["out"] for r in res.results], axis=0)
    return np.ascontiguousarray(out[:N])


if __name__ == "__main__":
    rng = np.random.default_rng(0)
    N, C, E = 2048, 128, 8192
    x = rng.standard_normal((N, C), dtype=np.float32)
    row = rng.integers(0, N, E).astype(np.int32)
    col = rng.integers(0, N, E).astype(np.int32)
    w = rng.random(E, dtype=np.float32)
    meta = _prep(x, row, col, w)
    print("tiles A:", meta["LT"], "tiles B:", meta["LTB"],
          "stages:", len(meta["gA"]["stages"]), len(meta["gB"]["stages"]))


# revision 16
# speedup vs baseline: 1.4919x; 1.0414x over previous
"""MixHop layer (gnn_message_passing) as a Trainium2 Bass/Tile SPMD kernel.

Math reformulation (A = sparse adjacency with edge weights, row=dst, col=src):
    x0 = x @ W0 + b0
    x1 = A @ (x @ W1 + b1) = (A @ x) @ W1 + d1 (x) b1      d1 = A @ 1
    x2 = A @ A @ (x @ W2 + b2) = (A @ A @ x) @ W2 + d2 (x) b2,  d2 = A @ d1
so only two sparse propagations of the raw features are needed:
    y1 = A @ x   (pass A),   y2 = A @ y1  (pass B, after all-gather of y1)

v3 design (driven by HW microbenchmarks):
  * dma_gather is descriptor-rate-bound (~5.3ns/desc shared wall), so pass A
    avoids gathers entirely: the host pre-expands x[col[e]] into dense bf16
    streams laid out partition-major; the device streams them with plain
    contiguous DMA.
  * pass B gathers y1 rows (256B bf16) from the all-gathered y1f.
  * All scatter matmuls run in bf16 (fp32 PSUM accumulate).  Selection
    matrices sel[e,r] = w[e]*(row[e]==r) are built per 128-edge tile on
    the Vector engine (tensor_scalar is_equal*mult) with a fraction
    offloaded to the idle Scalar engine as relu(-w*(iota-row)^2 + w).
  * Biases (1xb0, d1xb1, d2xb2) are K=1 rank-1 matmuls appended to the
    dense PSUM groups (PE has headroom; frees the Vector engine).
  * The AllGather of y1 (bf16) is chunked per stage-group and issued
    inline with pass A (pass A has no gpsimd work, so no head-of-line
    blocking); y1f is laid out stage-group-major so pass B's first
    source window is complete early.
  * pass B runs as two half-sweeps by source window: sweep 0 (first half
    of y1f) starts gathering while pass A is still finishing; per-block
    partial results are staged in SBUF and combined by per-block W2
    matmul groups at the end.
"""

import os
import sys

import numpy as np

for _p in ("/opt/trn_rl_repo",):
    if os.path.isdir(_p) and _p not in sys.path:
        sys.path.insert(0, _p)

import ml_dtypes

import concourse.bacc as bacc
import concourse.mybir as mybir
import concourse.tile as tile
from concourse.bass_utils import run_bass_kernel_spmd

F32 = mybir.dt.float32
BF16 = mybir.dt.bfloat16
I16 = mybir.dt.int16
NPBF = ml_dtypes.bfloat16

N_CORES = 8
P = 128          # partitions / rows per block / edges per tile
NCHUNK = 4       # source chunks (int16 index reach)
STAGE_TILE_CAP = 80   # max edge tiles staged in SBUF at once
SEL_ACT_EVERY = 4     # every k-th sel tile goes to the Scalar engine
AG_STAGES = 3         # stages per AllGather chunk


# ---------------------------------------------------------------------------
# host-side preprocessing
# ---------------------------------------------------------------------------

def _edge_grid(gblk, chunk, colrel, w, rloc, n_cores, B, nblk, nchunk):
    """Sort edges by (dest block, chunk); build the padded tile grid.

    Returns per-core tile grid metadata + per-edge slot assignment."""
    order = np.lexsort((chunk, gblk))
    gblk_s = gblk[order]
    chunk_s = chunk[order]
    col_s = colrel[order]
    w_s = w[order]
    rloc_s = rloc[order]

    E = len(gblk)
    grp = gblk_s * nchunk + chunk_s
    cnt = np.bincount(grp, minlength=nblk * nchunk)
    tiles = -(-cnt // P)
    T_BC = tiles.reshape(n_cores, B, nchunk).max(axis=0).astype(np.int64)
    empty = T_BC.sum(axis=1) == 0
    T_BC[empty, 0] = 1
    T_B = T_BC.sum(axis=1)
    LT = int(T_B.sum())

    stages = _make_stages(T_B)

    base_bc = np.zeros((B, nchunk), dtype=np.int64)
    pos0 = 0
    call_lens = []
    for (b0, nb, _) in stages:
        lens = []
        for c in range(nchunk):
            n = 0
            for b in range(b0, b0 + nb):
                base_bc[b, c] = pos0
                pos0 += T_BC[b, c]
                n += T_BC[b, c]
            lens.append(int(n))
        call_lens.append(lens)
    assert pos0 == LT

    grp_start = np.zeros(nblk * nchunk + 1, dtype=np.int64)
    np.cumsum(cnt, out=grp_start[1:])
    rank = np.arange(E, dtype=np.int64) - grp_start[grp]
    b_local = gblk_s % B
    e_core = gblk_s // B
    pos = P * base_bc[b_local, chunk_s] + rank

    L = P * LT
    emeta = np.zeros((n_cores, L, 4), dtype=np.float32)
    emeta[e_core, pos, 0] = rloc_s
    emeta[e_core, pos, 1] = -rloc_s
    emeta[e_core, pos, 2] = w_s
    emeta[e_core, pos, 3] = -w_s
    ecol = np.zeros((n_cores, L), dtype=np.int64)
    ecol[e_core, pos] = col_s
    used = np.zeros((n_cores, L), dtype=bool)
    used[e_core, pos] = True
    return dict(T_BC=T_BC, T_B=T_B, LT=LT, stages=stages,
                call_lens=call_lens, base_bc=base_bc, emeta=emeta,
                ecol=ecol, used=used)


def _prep(x, row, col, edge_weight, n_cores=N_CORES):
    N, C = x.shape
    S = -(-N // (n_cores * P)) * P          # shard rows per core
    NP = S * n_cores                        # padded node count
    B = S // P                              # dest blocks per core
    nblk = NP // P

    w64 = edge_weight.astype(np.float64)
    d1 = np.bincount(row, weights=w64, minlength=NP)[:NP]
    d2 = np.bincount(row, weights=w64 * d1[col], minlength=NP)[:NP]
    d1 = d1.astype(np.float32)
    d2 = d2.astype(np.float32)

    x16 = np.ascontiguousarray(x.astype(NPBF))

    gblk = (row // P).astype(np.int64)
    rloc = (row % P).astype(np.float32)
    w32 = edge_weight.astype(np.float32)
    col64 = col.astype(np.int64)

    # ---- pass A grid: no gathers -> no chunking at all ----
    gA = _edge_grid(gblk, np.zeros_like(gblk), col64, w32, rloc,
                    n_cores, B, nblk, 1)

    # dense edge-expanded features scaled by w[e], partition-major bf16.
    # Folding the weight into the features here makes the pass-A selection
    # matrix a pure one-hot (cheaper to build on DVE/ACT), and rounds the
    # w*x product to bf16 only once.
    LT = gA["LT"]
    xf32 = x.astype(np.float32)
    xeA = np.zeros((n_cores, P, LT * C), dtype=NPBF)
    for cix in range(n_cores):
        wslot = gA["emeta"][cix][:, 2]
        fe = (xf32[gA["ecol"][cix] % NP] * wslot[:, None]).astype(NPBF)
        xeA[cix] = np.ascontiguousarray(
            fe.reshape(LT, P, C).transpose(1, 0, 2).reshape(P, LT * C))

    # ---- AllGather groups: NG block ranges, each flat size <= 32768 ----
    NG = min(NCHUNK, B)
    nb_g = [B // NG + (1 if g < B % NG else 0) for g in range(NG)]
    blk0_g = np.concatenate([[0], np.cumsum(nb_g)]).astype(np.int64)
    CHW = [n_cores * nb * P for nb in nb_g]      # flat rows per window
    assert all(ch <= 32768 for ch in CHW)
    flat_off = np.concatenate([[0], np.cumsum(CHW)]).astype(np.int64)
    assert flat_off[-1] == NP

    flat_block_off = np.zeros((n_cores, B), dtype=np.int64)
    for g in range(NG):
        b0 = int(blk0_g[g])
        for cix in range(n_cores):
            for b in range(b0, b0 + nb_g[g]):
                flat_block_off[cix, b] = (flat_off[g]
                                          + cix * nb_g[g] * P
                                          + (b - b0) * P)

    # ---- pass B grid: chunk = AllGather window of the source ----
    src_core = col64 // S
    src_b = (col64 % S) // P
    src_r = col64 % P
    flat_src = (flat_block_off[src_core, src_b] + src_r).astype(np.int64)
    gwin = np.searchsorted(flat_off[1:], flat_src, side="right")
    winrel = flat_src - flat_off[gwin]
    gB = _edge_grid(gblk, gwin, winrel, w32, rloc, n_cores, B, nblk, NG)
    LTB = gB["LT"]
    eidxB = gB["ecol"].astype(np.int16)
    eidxB16 = np.ascontiguousarray(
        np.tile(eidxB.reshape(n_cores, (P * LTB) // 16, 16)
                .transpose(0, 2, 1), (1, 8, 1)))

    x_pad16 = np.zeros((NP, C), dtype=NPBF)
    x_pad16[:N] = x16
    xT = np.ascontiguousarray(
        x_pad16.reshape(n_cores, S, C).transpose(0, 2, 1))

    d1_sb = np.ascontiguousarray(d1.reshape(n_cores, S))
    d2_sb = np.ascontiguousarray(d2.reshape(n_cores, S))

    ch_half = -(-NG // 2)          # sweep 0 = windows [0, ch_half)

    return dict(N=N, C=C, S=S, NP=NP, B=B, NG=NG, nb_g=nb_g,
                blk0_g=blk0_g, CHW=CHW, ch_half=ch_half,
                gA=gA, gB=gB, LT=LT, LTB=LTB,
                xeA=xeA, eidxB16=eidxB16, xT=xT, d1=d1_sb, d2=d2_sb)


def _make_stages(T_B, cap=STAGE_TILE_CAP):
    stages = []
    b = 0
    off = 0
    B = len(T_B)
    while b < B:
        start = b
        soff = off
        tot = 0
        while b < B and (b == start or tot + T_B[b] <= cap):
            tot += int(T_B[b])
            off += int(T_B[b])
            b += 1
        stages.append((start, b - start, soff))
    return stages


# ---------------------------------------------------------------------------
# device program
# ---------------------------------------------------------------------------

def build_program(meta, n_cores=N_CORES):
    C, S, NP, B = meta["C"], meta["S"], meta["NP"], meta["B"]
    NG, nb_g, blk0_g, CHW = (meta["NG"], meta["nb_g"], meta["blk0_g"],
                             meta["CHW"])
    ch_half = meta["ch_half"]
    gA, gB = meta["gA"], meta["gB"]
    LT, LTB = meta["LT"], meta["LTB"]
    stages = gA["stages"]
    stagesB = gB["stages"]
    ts_max = max(
        max(int(gA["T_B"][b0:b0 + nb].sum()) for b0, nb, _ in stages),
        max(int(gB["T_B"][b0:b0 + nb].sum()) for b0, nb, _ in stagesB))
    NBMAX = max(max(nb for _, nb, _ in stages),
                max(nb for _, nb, _ in stagesB))

    nc = bacc.Bacc("TRN2", target_bir_lowering=False, debug=False,
                   num_devices=n_cores, num_swdge_queues=4)

    xeA_d = nc.dram_tensor("xeA", [P, LT * C], BF16, kind="ExternalInput")
    emeta_d = nc.dram_tensor("emeta", [P, LT, 4], F32, kind="ExternalInput")
    emetaB_d = nc.dram_tensor("emetaB", [P, LTB, 4], F32,
                              kind="ExternalInput")
    eidxB_d = nc.dram_tensor("eidxB", [P, (P * LTB) // 16], I16,
                             kind="ExternalInput")
    xT_d = nc.dram_tensor("xT", [C, S], BF16, kind="ExternalInput")
    wmat_d = nc.dram_tensor("wmat", [C, 3 * C], BF16, kind="ExternalInput")
    consts_d = nc.dram_tensor("consts", [P, 2 * P], BF16,
                              kind="ExternalInput")
    consts32_d = nc.dram_tensor("consts32", [P, 3 * C + 2 * B], F32,
                                kind="ExternalInput")
    out_d = nc.dram_tensor("out", [S, 3 * C], F32, kind="ExternalOutput")

    sel_ctr = [0]

    with tile.TileContext(nc) as tc:
        with (
            tc.tile_pool(name="cpool", bufs=1) as cpool,
            tc.tile_pool(name="fpool", bufs=3) as fpool,
            tc.tile_pool(name="fpoolB", bufs=3) as fpoolB,
            tc.tile_pool(name="mpool", bufs=3) as mpool,
            tc.tile_pool(name="mpoolB", bufs=3) as mpoolB,
            tc.tile_pool(name="spool", bufs=10) as spool,
            tc.tile_pool(name="zpool", bufs=4) as zpool,
            tc.tile_pool(name="vpool", bufs=2) as vpool,
            tc.tile_pool(name="ypsum", bufs=3, space="PSUM") as ypsum,
            tc.tile_pool(name="tpsum", bufs=2, space="PSUM") as tpsum,
            tc.tile_pool(name="xpsum", bufs=3, space="PSUM") as xpsum,
            tc.tile_pool(name="dram2", bufs=1, space="DRAM") as dram,
        ):
            y1s = dram.tile([S, C], BF16)
            y1fg = []
            for g in range(NG):
                y1fg_g = dram.tile([CHW[g], C], BF16, addr_space="Shared",
                                   name=f"y1fg{g}")
                y1fg.append(y1fg_g)

            consts_t = cpool.tile([P, 2 * P], BF16, tag="consts")
            nc.sync.dma_start(consts_t[:], consts_d[:])
            iota_t = consts_t[:, 0 * P:1 * P]
            eye_t = consts_t[:, 1 * P:2 * P]
            consts32_t = cpool.tile([P, 3 * C + 2 * B], F32, tag="c32")
            nc.sync.dma_start(consts32_t[:], consts32_d[:])
            b0b = consts32_t[:, 0:C]
            b1b = consts32_t[:, C:2 * C]
            b2b = consts32_t[:, 2 * C:3 * C]
            d1col = consts32_t[:, 3 * C:3 * C + B]
            d2col = consts32_t[:, 3 * C + B:3 * C + 2 * B]
            wmat_t = cpool.tile([C, 3 * C], BF16, tag="wmat")
            nc.sync.dma_start(wmat_t[:], wmat_d[:])
            w0_t = wmat_t[:, 0 * C:1 * C]
            w1_t = wmat_t[:, 1 * C:2 * C]
            w2_t = wmat_t[:, 2 * C:3 * C]
            ystg0 = cpool.tile([P, B, C], BF16, tag="ystg0")
            ystg1 = cpool.tile([P, B, C], BF16, tag="ystg1")
            ystg = [ystg0, ystg1]
            nc.vector.memset(ystg[0][:], 0.0)
            nc.vector.memset(ystg[1][:], 0.0)

            def make_sel(row_ap, nrow_ap, w_ap, nw_ap, act_every,
                         weighted=True):
                """sel[e,r]: one-hot row pattern, scaled by w if weighted."""
                sel_ctr[0] += 1
                sel = spool.tile([P, P], BF16, tag="sel")
                if sel_ctr[0] % act_every == 0:
                    z2 = zpool.tile([P, P], BF16, tag="z2")
                    nc.scalar.activation(
                        out=z2[:], in_=iota_t,
                        func=mybir.ActivationFunctionType.Square,
                        bias=nrow_ap, scale=1.0)
                    if weighted:
                        nc.scalar.activation(
                            out=sel[:], in_=z2[:],
                            func=mybir.ActivationFunctionType.Relu,
                            bias=w_ap, scale=nw_ap)
                    else:
                        nc.scalar.activation(
                            out=sel[:], in_=z2[:],
                            func=mybir.ActivationFunctionType.Relu,
                            bias=1.0, scale=-1.0)
                elif weighted:
                    nc.vector.tensor_scalar(
                        out=sel[:], in0=iota_t,
                        scalar1=row_ap, scalar2=w_ap,
                        op0=mybir.AluOpType.is_equal,
                        op1=mybir.AluOpType.mult)
                else:
                    nc.vector.tensor_scalar(
                        out=sel[:], in0=iota_t,
                        scalar1=row_ap, scalar2=None,
                        op0=mybir.AluOpType.is_equal)
                return sel

            # ---------------- pass A (+ inline chunked AllGather) --------
            def emit_ag(g):
                r0 = int(blk0_g[g]) * P
                rows = nb_g[g] * P
                nc.gpsimd.collective_compute(
                    "AllGather",
                    mybir.AluOpType.bypass,
                    replica_groups=[list(range(n_cores))],
                    ins=[y1s[r0:r0 + rows, :].opt()],
                    outs=[y1fg[g][:].opt()],
                )

            # AG windows 0..NG-2 fire inline in pass A right after their
            # blocks complete (pass A has no other gpsimd work, so the
            # blocking collective dispatch hides under pass-A compute).
            # The last window is dispatched a few stages into sweep 0:
            # its y1s rows are complete by then, and the already-issued
            # sweep-0 gather calls keep draining under its transfer.
            n_inline = max(1, NG - 1)
            ag_after_stage = {}
            for g in range(n_inline):
                lb = int(blk0_g[g + 1]) - 1
                for si, (b0, nb, _) in enumerate(stages):
                    if b0 <= lb < b0 + nb:
                        ag_after_stage.setdefault(si, []).append(g)
                        break

            for si, (b0, nb, soff) in enumerate(stages):
                ts = int(gA["T_B"][b0:b0 + nb].sum())
                mrow = mpool.tile([P, ts_max, 4], F32, tag="meta")
                nc.sync.dma_start(mrow[:, :ts, :],
                                  emeta_d[:, soff:soff + ts, :])
                f_t = fpool.tile([P, ts_max, C], BF16, tag="f")
                nc.sync.dma_start(
                    f_t[:, :ts, :],
                    xeA_d[:, soff * C:(soff + ts) * C]
                    .rearrange("p (t c) -> p t c", c=C))
                y_st = vpool.tile([P, NBMAX, C], BF16, tag="yst")
                for b in range(b0, b0 + nb):
                    kk = b - b0
                    y_ps = ypsum.tile([P, C], F32, tag="ypsum")
                    g0 = int(gA["base_bc"][b, 0] - soff)
                    ntile = int(gA["T_BC"][b, 0])
                    for k in range(ntile):
                        t = g0 + k
                        sel = make_sel(mrow[:, t, 0:1], mrow[:, t, 1:2],
                                       mrow[:, t, 2:3], mrow[:, t, 3:4],
                                       act_every=5, weighted=False)
                        nc.tensor.matmul(
                            out=y_ps[:], lhsT=sel[:], rhs=f_t[:, t, :],
                            start=(k == 0), stop=(k == ntile - 1))
                    nc.vector.tensor_copy(y_st[:, kk, :], y_ps[:])
                nc.sync.dma_start(
                    y1s[b0 * P:(b0 + nb) * P, :]
                    .rearrange("(g p) c -> p g c", p=P),
                    y_st[:, :nb, :])
                for g in ag_after_stage.get(si, []):
                    emit_ag(g)

            # ---------------- dense x0/x1 loops (overlap AG) -------------
            def emit_x0_x1(gsz=4):
                for g0 in range(0, B, gsz):
                    gn = min(gsz, B - g0)
                    xT_t = vpool.tile([C, gsz * P], BF16, tag="xT")
                    nc.sync.dma_start(xT_t[:, :gn * P],
                                      xT_d[:, g0 * P:(g0 + gn) * P])
                    y_ld = vpool.tile([P, gsz, C], BF16, tag="yld")
                    nc.sync.dma_start(
                        y_ld[:, :gn, :],
                        y1s[g0 * P:(g0 + gn) * P, :]
                        .rearrange("(g p) c -> p g c", p=P))
                    x01_st = vpool.tile([P, gsz, 2 * C], F32, tag="x01st")
                    for k in range(gn):
                        b = g0 + k
                        x0_ps = xpsum.tile([P, C], F32, tag="xpsum")
                        nc.tensor.matmul(
                            out=x0_ps[:], lhsT=xT_t[:, k * P:(k + 1) * P],
                            rhs=w0_t, start=True, stop=True)
                        nc.vector.tensor_tensor(
                            out=x01_st[:, k, 0:C], in0=x0_ps[:], in1=b0b,
                            op=mybir.AluOpType.add)
                        yT_ps = tpsum.tile([P, C], BF16, tag="tpsum")
                        nc.tensor.transpose(yT_ps[:], y_ld[:, k, :], eye_t)
                        yT_sb = vpool.tile([P, C], BF16, tag="ytsb")
                        nc.scalar.activation(
                            out=yT_sb[:], in_=yT_ps[:],
                            func=mybir.ActivationFunctionType.Copy,
                            scale=1.0)
                        x1_ps = xpsum.tile([P, C], F32, tag="xpsum")
                        nc.tensor.matmul(out=x1_ps[:], lhsT=yT_sb[:],
                                         rhs=w1_t, start=True, stop=True)
                        tmp1 = zpool.tile([P, C], F32, tag="tmp")
                        nc.scalar.activation(
                            out=tmp1[:], in_=b1b,
                            func=mybir.ActivationFunctionType.Copy,
                            scale=d1col[:, b:b + 1])
                        nc.vector.tensor_tensor(
                            out=x01_st[:, k, C:2 * C], in0=x1_ps[:],
                            in1=tmp1[:], op=mybir.AluOpType.add)
                    nc.sync.dma_start(
                        out_d[g0 * P:(g0 + gn) * P, 0:2 * C]
                        .rearrange("(g p) c -> p g c", p=P),
                        x01_st[:, :gn, :])

            # ---------------- pass B: one sweep per source window ---------
            def emit_x2_blocks(b0, nb):
                x2_st = vpool.tile([P, NBMAX, C], F32, tag="x2st")
                for b in range(b0, b0 + nb):
                    kk = b - b0
                    x_ps = xpsum.tile([P, C], F32, tag="xpsum")
                    nc.tensor.matmul(out=x_ps[:], lhsT=ystg[0][:, b, :],
                                     rhs=w2_t, start=True, stop=False)
                    nc.tensor.matmul(out=x_ps[:], lhsT=ystg[1][:, b, :],
                                     rhs=w2_t, start=False, stop=True)
                    tmp2 = zpool.tile([P, C], F32, tag="tmp")
                    nc.scalar.activation(
                        out=tmp2[:], in_=b2b,
                        func=mybir.ActivationFunctionType.Copy,
                        scale=d2col[:, b:b + 1])
                    nc.vector.tensor_tensor(
                        out=x2_st[:, kk, :], in0=x_ps[:],
                        in1=tmp2[:], op=mybir.AluOpType.add)
                nc.sync.dma_start(
                    out_d[b0 * P:(b0 + nb) * P, 2 * C:3 * C]
                    .rearrange("(g p) c -> p g c", p=P),
                    x2_st[:, :nb, :])

            def emit_passB_sweep(sw):
                clo = 0 if sw == 0 else ch_half
                chi = ch_half if sw == 0 else NG
                last = (sw == 1)
                for si, (b0, nb, soff) in enumerate(stagesB):
                    if sw == 0 and si == min(3, len(stagesB) - 1):
                        for g in range(n_inline, NG):
                            emit_ag(g)
                    lens = gB["call_lens"][si]
                    rel0 = sum(lens[:clo])
                    ts_sw = sum(lens[clo:chi])
                    if ts_sw == 0:
                        if last:
                            emit_x2_blocks(b0, nb)
                        continue
                    so = soff + rel0
                    mrow = mpoolB.tile([P, ts_max, 4], F32, tag="metaB")
                    nc.sync.dma_start(mrow[:, :ts_sw, :],
                                      emetaB_d[:, so:so + ts_sw, :])
                    idx_t = mpoolB.tile([P, (P * ts_max) // 16], I16,
                                        tag="idxB")
                    i16o = (P * so) // 16
                    i16n = (P * ts_sw) // 16
                    nc.sync.dma_start(idx_t[:, :i16n],
                                      eidxB_d[:, i16o:i16o + i16n])
                    f_t = fpoolB.tile([P, ts_max, C], BF16, tag="fB")
                    rel = 0
                    for c in range(clo, chi):
                        tsc = lens[c]
                        if tsc == 0:
                            continue
                        nidx = P * tsc
                        nc.gpsimd.dma_gather(
                            out_ap=f_t[:, rel:rel + tsc, :],
                            in_ap=y1fg[c][:, :],
                            idxs_ap=idx_t[:, (P * rel) // 16:
                                          (P * rel) // 16 + nidx // 16],
                            num_idxs=nidx,
                            num_idxs_reg=nidx,
                            elem_size=C,
                            single_packet=False,
                            queue_num=(si + c) % 4,
                        )
                        rel += tsc
                    for b in range(b0, b0 + nb):
                        tl = [(int(gB["base_bc"][b, c] - so),
                               int(gB["T_BC"][b, c]))
                              for c in range(clo, chi)
                              if gB["T_BC"][b, c] > 0]
                        ntile = sum(n for _, n in tl)
                        if ntile == 0:
                            continue
                        y_ps = ypsum.tile([P, C], F32, tag="ypsum")
                        k = 0
                        for (g0, n) in tl:
                            for t in range(g0, g0 + n):
                                sel = make_sel(mrow[:, t, 0:1],
                                               mrow[:, t, 1:2],
                                               mrow[:, t, 2:3],
                                               mrow[:, t, 3:4],
                                               act_every=SEL_ACT_EVERY)
                                nc.tensor.matmul(
                                    out=y_ps[:], lhsT=f_t[:, t, :],
                                    rhs=sel[:],
                                    start=(k == 0), stop=(k == ntile - 1))
                                k += 1
                        nc.scalar.activation(
                            out=ystg[sw][:, b, :], in_=y_ps[:],
                            func=mybir.ActivationFunctionType.Copy,
                            scale=1.0)
                    if last:
                        emit_x2_blocks(b0, nb)

            emit_x0_x1()
            emit_passB_sweep(0)
            emit_passB_sweep(1)

    nc.compile()
    return nc


# ---------------------------------------------------------------------------
# entry point
# ---------------------------------------------------------------------------

def make_in_maps(meta, W0, b0, W1, b1, W2, b2, n_cores=N_CORES):
    C, S = meta["C"], meta["S"]
    iota = np.tile(np.arange(P, dtype=np.float32), (P, 1)).astype(NPBF)
    eye = np.eye(P, dtype=np.float32).astype(NPBF)
    consts = np.ascontiguousarray(np.concatenate([iota, eye], axis=1))
    wmat = np.concatenate(
        [np.asarray(W0, np.float32), np.asarray(W1, np.float32),
         np.asarray(W2, np.float32)], axis=1).astype(NPBF)
    B = meta["B"]
    b0b = np.tile(np.asarray(b0, np.float32), (P, 1))
    b1b = np.tile(np.asarray(b1, np.float32), (P, 1))
    b2b = np.tile(np.asarray(b2, np.float32), (P, 1))
    in_maps = []
    for c in range(n_cores):
        d1c = np.ascontiguousarray(
            meta["d1"][c].reshape(B, P).T)          # [128, B]
        d2c = np.ascontiguousarray(
            meta["d2"][c].reshape(B, P).T)
        consts32 = np.ascontiguousarray(np.concatenate(
            [b0b, b1b, b2b, d1c, d2c], axis=1))
        in_maps.append({
            "xeA": meta["xeA"][c],
            "emeta": _meta_tile(meta["gA"]["emeta"][c], meta["LT"]),
            "emetaB": _meta_tile(meta["gB"]["emeta"][c], meta["LTB"]),
            "eidxB": meta["eidxB16"][c],
            "xT": meta["xT"][c],
            "wmat": wmat,
            "consts": consts,
            "consts32": consts32,
        })
    return in_maps


def _meta_tile(em, LT_):
    # [L, 4] edge-order -> [128, LT, 4]: edge j -> partition j%128, tile j//128
    return np.ascontiguousarray(em.reshape(LT_, P, 4).transpose(1, 0, 2))


def kernel(x, row, col, edge_weight, W0, b0, W1, b1, W2, b2):
    x = np.asarray(x, np.float32)
    row = np.asarray(row, np.int32)
    col = np.asarray(col, np.int32)
    edge_weight = np.asarray(edge_weight, np.float32)
    N = x.shape[0]

    meta = _prep(x, row, col, edge_weight)
    nc = build_program(meta)
    in_maps = make_in_maps(meta, W0, b0, W1, b1, W2, b2)
    res = run_bass_kernel_spmd(nc, in_maps, core_ids=list(range(N_CORES)))
    out = np.concatenate([r["out"] for r in res.results], axis=0)
    return np.ascontiguousarray(out[:N])


if __name__ == "__main__":
    rng = np.random.default_rng(0)
    N, C, E = 2048, 128, 8192
    x = rng.standard_normal((N, C), dtype=np.float32)
    row = rng.integers(0, N, E).astype(np.int32)
    col = rng.integers(0, N, E).astype(np.int32)
    w = rng.random(E, dtype=np.float32)
    meta = _prep(x, row, col, w)
    print("tiles A:", meta["LT"], "tiles B:", meta["LTB"],
          "stages:", len(meta["gA"]["stages"]), len(meta["gB"]["stages"]))


# revision 17
# speedup vs baseline: 1.5117x; 1.0133x over previous
"""MixHop layer (gnn_message_passing) as a Trainium2 Bass/Tile SPMD kernel.

Math reformulation (A = sparse adjacency with edge weights, row=dst, col=src):
    x0 = x @ W0 + b0
    x1 = A @ (x @ W1 + b1) = (A @ x) @ W1 + d1 (x) b1      d1 = A @ 1
    x2 = A @ A @ (x @ W2 + b2) = (A @ A @ x) @ W2 + d2 (x) b2,  d2 = A @ d1
so only two sparse propagations of the raw features are needed:
    y1 = A @ x   (pass A),   y2 = A @ y1  (pass B, after all-gather of y1)

v3 design (driven by HW microbenchmarks):
  * dma_gather is descriptor-rate-bound (~5.3ns/desc shared wall), so pass A
    avoids gathers entirely: the host pre-expands x[col[e]] into dense bf16
    streams laid out partition-major; the device streams them with plain
    contiguous DMA.
  * pass B gathers y1 rows (256B bf16) from the all-gathered y1f.
  * All scatter matmuls run in bf16 (fp32 PSUM accumulate).  Selection
    matrices sel[e,r] = w[e]*(row[e]==r) are built per 128-edge tile on
    the Vector engine (tensor_scalar is_equal*mult) with a fraction
    offloaded to the idle Scalar engine as relu(-w*(iota-row)^2 + w).
  * Biases (1xb0, d1xb1, d2xb2) are K=1 rank-1 matmuls appended to the
    dense PSUM groups (PE has headroom; frees the Vector engine).
  * The AllGather of y1 (bf16) is chunked per stage-group and issued
    inline with pass A (pass A has no gpsimd work, so no head-of-line
    blocking); y1f is laid out stage-group-major so pass B's first
    source window is complete early.
  * pass B runs as two half-sweeps by source window: sweep 0 (first half
    of y1f) starts gathering while pass A is still finishing; per-block
    partial results are staged in SBUF and combined by per-block W2
    matmul groups at the end.
"""

import os
import sys

import numpy as np

for _p in ("/opt/trn_rl_repo",):
    if os.path.isdir(_p) and _p not in sys.path:
        sys.path.insert(0, _p)

import ml_dtypes

import concourse.bacc as bacc
import concourse.mybir as mybir
import concourse.tile as tile
from concourse.bass_utils import run_bass_kernel_spmd

F32 = mybir.dt.float32
BF16 = mybir.dt.bfloat16
I16 = mybir.dt.int16
NPBF = ml_dtypes.bfloat16

N_CORES = 8
P = 128          # partitions / rows per block / edges per tile
NCHUNK = 4       # source chunks (int16 index reach)
STAGE_TILE_CAP = 80   # max edge tiles staged in SBUF at once
SEL_ACT_EVERY = 4     # every k-th sel tile goes to the Scalar engine
AG_STAGES = 3         # stages per AllGather chunk


# ---------------------------------------------------------------------------
# host-side preprocessing
# ---------------------------------------------------------------------------

def _edge_grid(gblk, chunk, colrel, w, rloc, n_cores, B, nblk, nchunk):
    """Sort edges by (dest block, chunk); build the padded tile grid.

    Returns per-core tile grid metadata + per-edge slot assignment."""
    order = np.lexsort((chunk, gblk))
    gblk_s = gblk[order]
    chunk_s = chunk[order]
    col_s = colrel[order]
    w_s = w[order]
    rloc_s = rloc[order]

    E = len(gblk)
    grp = gblk_s * nchunk + chunk_s
    cnt = np.bincount(grp, minlength=nblk * nchunk)
    tiles = -(-cnt // P)
    T_BC = tiles.reshape(n_cores, B, nchunk).max(axis=0).astype(np.int64)
    empty = T_BC.sum(axis=1) == 0
    T_BC[empty, 0] = 1
    T_B = T_BC.sum(axis=1)
    LT = int(T_B.sum())

    stages = _make_stages(T_B)

    base_bc = np.zeros((B, nchunk), dtype=np.int64)
    pos0 = 0
    call_lens = []
    for (b0, nb, _) in stages:
        lens = []
        for c in range(nchunk):
            n = 0
            for b in range(b0, b0 + nb):
                base_bc[b, c] = pos0
                pos0 += T_BC[b, c]
                n += T_BC[b, c]
            lens.append(int(n))
        call_lens.append(lens)
    assert pos0 == LT

    grp_start = np.zeros(nblk * nchunk + 1, dtype=np.int64)
    np.cumsum(cnt, out=grp_start[1:])
    rank = np.arange(E, dtype=np.int64) - grp_start[grp]
    b_local = gblk_s % B
    e_core = gblk_s // B
    pos = P * base_bc[b_local, chunk_s] + rank

    L = P * LT
    emeta = np.zeros((n_cores, L, 4), dtype=np.float32)
    emeta[e_core, pos, 0] = rloc_s
    emeta[e_core, pos, 1] = -rloc_s
    emeta[e_core, pos, 2] = w_s
    emeta[e_core, pos, 3] = -w_s
    ecol = np.zeros((n_cores, L), dtype=np.int64)
    ecol[e_core, pos] = col_s
    used = np.zeros((n_cores, L), dtype=bool)
    used[e_core, pos] = True
    return dict(T_BC=T_BC, T_B=T_B, LT=LT, stages=stages,
                call_lens=call_lens, base_bc=base_bc, emeta=emeta,
                ecol=ecol, used=used)


def _prep(x, row, col, edge_weight, n_cores=N_CORES):
    N, C = x.shape
    S = -(-N // (n_cores * P)) * P          # shard rows per core
    NP = S * n_cores                        # padded node count
    B = S // P                              # dest blocks per core
    nblk = NP // P

    w64 = edge_weight.astype(np.float64)
    d1 = np.bincount(row, weights=w64, minlength=NP)[:NP]
    d2 = np.bincount(row, weights=w64 * d1[col], minlength=NP)[:NP]
    d1 = d1.astype(np.float32)
    d2 = d2.astype(np.float32)

    x16 = np.ascontiguousarray(x.astype(NPBF))

    gblk = (row // P).astype(np.int64)
    rloc = (row % P).astype(np.float32)
    w32 = edge_weight.astype(np.float32)
    col64 = col.astype(np.int64)

    # ---- pass A grid: no gathers -> no chunking at all ----
    gA = _edge_grid(gblk, np.zeros_like(gblk), col64, w32, rloc,
                    n_cores, B, nblk, 1)

    # dense edge-expanded features scaled by w[e], partition-major bf16.
    # Folding the weight into the features here makes the pass-A selection
    # matrix a pure one-hot (cheaper to build on DVE/ACT), and rounds the
    # w*x product to bf16 only once.
    LT = gA["LT"]
    xf32 = x.astype(np.float32)
    xeA = np.zeros((n_cores, P, LT * C), dtype=NPBF)
    for cix in range(n_cores):
        wslot = gA["emeta"][cix][:, 2]
        fe = (xf32[gA["ecol"][cix] % NP] * wslot[:, None]).astype(NPBF)
        xeA[cix] = np.ascontiguousarray(
            fe.reshape(LT, P, C).transpose(1, 0, 2).reshape(P, LT * C))

    # ---- AllGather groups: NG block ranges, each flat size <= 32768 ----
    NG = min(NCHUNK, B)
    nb_g = [B // NG + (1 if g < B % NG else 0) for g in range(NG)]
    blk0_g = np.concatenate([[0], np.cumsum(nb_g)]).astype(np.int64)
    CHW = [n_cores * nb * P for nb in nb_g]      # flat rows per window
    assert all(ch <= 32768 for ch in CHW)
    flat_off = np.concatenate([[0], np.cumsum(CHW)]).astype(np.int64)
    assert flat_off[-1] == NP

    flat_block_off = np.zeros((n_cores, B), dtype=np.int64)
    for g in range(NG):
        b0 = int(blk0_g[g])
        for cix in range(n_cores):
            for b in range(b0, b0 + nb_g[g]):
                flat_block_off[cix, b] = (flat_off[g]
                                          + cix * nb_g[g] * P
                                          + (b - b0) * P)

    # ---- pass B grid: chunk = AllGather window of the source ----
    src_core = col64 // S
    src_b = (col64 % S) // P
    src_r = col64 % P
    flat_src = (flat_block_off[src_core, src_b] + src_r).astype(np.int64)
    gwin = np.searchsorted(flat_off[1:], flat_src, side="right")
    winrel = flat_src - flat_off[gwin]
    gB = _edge_grid(gblk, gwin, winrel, w32, rloc, n_cores, B, nblk, NG)
    LTB = gB["LT"]
    eidxB = gB["ecol"].astype(np.int16)
    eidxB16 = np.ascontiguousarray(
        np.tile(eidxB.reshape(n_cores, (P * LTB) // 16, 16)
                .transpose(0, 2, 1), (1, 8, 1)))

    x_pad16 = np.zeros((NP, C), dtype=NPBF)
    x_pad16[:N] = x16
    xT = np.ascontiguousarray(
        x_pad16.reshape(n_cores, S, C).transpose(0, 2, 1))

    d1_sb = np.ascontiguousarray(d1.reshape(n_cores, S))
    d2_sb = np.ascontiguousarray(d2.reshape(n_cores, S))

    ch_half = -(-NG // 2)          # sweep 0 = windows [0, ch_half)

    return dict(N=N, C=C, S=S, NP=NP, B=B, NG=NG, nb_g=nb_g,
                blk0_g=blk0_g, CHW=CHW, ch_half=ch_half,
                gA=gA, gB=gB, LT=LT, LTB=LTB,
                xeA=xeA, eidxB16=eidxB16, xT=xT, d1=d1_sb, d2=d2_sb)


def _make_stages(T_B, cap=STAGE_TILE_CAP):
    stages = []
    b = 0
    off = 0
    B = len(T_B)
    while b < B:
        start = b
        soff = off
        tot = 0
        while b < B and (b == start or tot + T_B[b] <= cap):
            tot += int(T_B[b])
            off += int(T_B[b])
            b += 1
        stages.append((start, b - start, soff))
    return stages


# ---------------------------------------------------------------------------
# device program
# ---------------------------------------------------------------------------

def build_program(meta, n_cores=N_CORES):
    C, S, NP, B = meta["C"], meta["S"], meta["NP"], meta["B"]
    NG, nb_g, blk0_g, CHW = (meta["NG"], meta["nb_g"], meta["blk0_g"],
                             meta["CHW"])
    ch_half = meta["ch_half"]
    gA, gB = meta["gA"], meta["gB"]
    LT, LTB = meta["LT"], meta["LTB"]
    stages = gA["stages"]
    stagesB = gB["stages"]
    ts_max = max(
        max(int(gA["T_B"][b0:b0 + nb].sum()) for b0, nb, _ in stages),
        max(int(gB["T_B"][b0:b0 + nb].sum()) for b0, nb, _ in stagesB))
    NBMAX = max(max(nb for _, nb, _ in stages),
                max(nb for _, nb, _ in stagesB))

    nc = bacc.Bacc("TRN2", target_bir_lowering=False, debug=False,
                   num_devices=n_cores, num_swdge_queues=4)

    xeA_d = nc.dram_tensor("xeA", [P, LT * C], BF16, kind="ExternalInput")
    emeta_d = nc.dram_tensor("emeta", [P, LT, 4], F32, kind="ExternalInput")
    emetaB_d = nc.dram_tensor("emetaB", [P, LTB, 4], F32,
                              kind="ExternalInput")
    eidxB_d = nc.dram_tensor("eidxB", [P, (P * LTB) // 16], I16,
                             kind="ExternalInput")
    xT_d = nc.dram_tensor("xT", [C, S], BF16, kind="ExternalInput")
    wmat_d = nc.dram_tensor("wmat", [C, 3 * C], BF16, kind="ExternalInput")
    consts_d = nc.dram_tensor("consts", [P, 2 * P], BF16,
                              kind="ExternalInput")
    consts32_d = nc.dram_tensor("consts32", [P, 3 * C + 2 * B], F32,
                                kind="ExternalInput")
    out_d = nc.dram_tensor("out", [S, 3 * C], F32, kind="ExternalOutput")

    sel_ctr = [0]

    with tile.TileContext(nc) as tc:
        with (
            tc.tile_pool(name="cpool", bufs=1) as cpool,
            tc.tile_pool(name="fpool", bufs=3) as fpool,
            tc.tile_pool(name="fpoolB", bufs=3) as fpoolB,
            tc.tile_pool(name="mpool", bufs=3) as mpool,
            tc.tile_pool(name="mpoolB", bufs=3) as mpoolB,
            tc.tile_pool(name="spool", bufs=10) as spool,
            tc.tile_pool(name="zpool", bufs=4) as zpool,
            tc.tile_pool(name="vpool", bufs=2) as vpool,
            tc.tile_pool(name="ypsum", bufs=3, space="PSUM") as ypsum,
            tc.tile_pool(name="tpsum", bufs=2, space="PSUM") as tpsum,
            tc.tile_pool(name="xpsum", bufs=3, space="PSUM") as xpsum,
            tc.tile_pool(name="dram2", bufs=1, space="DRAM") as dram,
        ):
            y1s = dram.tile([S, C], BF16)
            y1fg = []
            for g in range(NG):
                y1fg_g = dram.tile([CHW[g], C], BF16, addr_space="Shared",
                                   name=f"y1fg{g}")
                y1fg.append(y1fg_g)

            consts_t = cpool.tile([P, 2 * P], BF16, tag="consts")
            nc.sync.dma_start(consts_t[:], consts_d[:])
            iota_t = consts_t[:, 0 * P:1 * P]
            eye_t = consts_t[:, 1 * P:2 * P]
            consts32_t = cpool.tile([P, 3 * C + 2 * B], F32, tag="c32")
            nc.sync.dma_start(consts32_t[:], consts32_d[:])
            b0b = consts32_t[:, 0:C]
            b1b = consts32_t[:, C:2 * C]
            b2b = consts32_t[:, 2 * C:3 * C]
            d1col = consts32_t[:, 3 * C:3 * C + B]
            d2col = consts32_t[:, 3 * C + B:3 * C + 2 * B]
            wmat_t = cpool.tile([C, 3 * C], BF16, tag="wmat")
            nc.sync.dma_start(wmat_t[:], wmat_d[:])
            w0_t = wmat_t[:, 0 * C:1 * C]
            w1_t = wmat_t[:, 1 * C:2 * C]
            w2_t = wmat_t[:, 2 * C:3 * C]
            ystg0 = cpool.tile([P, B, C], BF16, tag="ystg0")
            ystg1 = cpool.tile([P, B, C], BF16, tag="ystg1")
            ystg = [ystg0, ystg1]
            nc.vector.memset(ystg[0][:], 0.0)
            nc.vector.memset(ystg[1][:], 0.0)

            def make_sel(row_ap, nrow_ap, w_ap, nw_ap, act_every,
                         weighted=True):
                """sel[e,r]: one-hot row pattern, scaled by w if weighted."""
                sel_ctr[0] += 1
                sel = spool.tile([P, P], BF16, tag="sel")
                if sel_ctr[0] % act_every == 0:
                    z2 = zpool.tile([P, P], BF16, tag="z2")
                    nc.scalar.activation(
                        out=z2[:], in_=iota_t,
                        func=mybir.ActivationFunctionType.Square,
                        bias=nrow_ap, scale=1.0)
                    if weighted:
                        nc.scalar.activation(
                            out=sel[:], in_=z2[:],
                            func=mybir.ActivationFunctionType.Relu,
                            bias=w_ap, scale=nw_ap)
                    else:
                        nc.scalar.activation(
                            out=sel[:], in_=z2[:],
                            func=mybir.ActivationFunctionType.Relu,
                            bias=1.0, scale=-1.0)
                elif weighted:
                    nc.vector.tensor_scalar(
                        out=sel[:], in0=iota_t,
                        scalar1=row_ap, scalar2=w_ap,
                        op0=mybir.AluOpType.is_equal,
                        op1=mybir.AluOpType.mult)
                else:
                    nc.vector.tensor_scalar(
                        out=sel[:], in0=iota_t,
                        scalar1=row_ap, scalar2=None,
                        op0=mybir.AluOpType.is_equal)
                return sel

            # ---------------- pass A (+ inline chunked AllGather) --------
            def emit_ag(g):
                r0 = int(blk0_g[g]) * P
                rows = nb_g[g] * P
                nc.gpsimd.collective_compute(
                    "AllGather",
                    mybir.AluOpType.bypass,
                    replica_groups=[list(range(n_cores))],
                    ins=[y1s[r0:r0 + rows, :].opt()],
                    outs=[y1fg[g][:].opt()],
                )

            # AG windows 0..NG-2 fire inline in pass A right after their
            # blocks complete (pass A has no other gpsimd work, so the
            # blocking collective dispatch hides under pass-A compute).
            # The last window is dispatched a few stages into sweep 0:
            # its y1s rows are complete by then, and the already-issued
            # sweep-0 gather calls keep draining under its transfer.
            n_inline = max(1, NG - 2)
            ag_after_stage = {}
            for g in range(n_inline):
                lb = int(blk0_g[g + 1]) - 1
                for si, (b0, nb, _) in enumerate(stages):
                    if b0 <= lb < b0 + nb:
                        ag_after_stage.setdefault(si, []).append(g)
                        break

            for si, (b0, nb, soff) in enumerate(stages):
                ts = int(gA["T_B"][b0:b0 + nb].sum())
                mrow = mpool.tile([P, ts_max, 4], F32, tag="meta")
                nc.sync.dma_start(mrow[:, :ts, :],
                                  emeta_d[:, soff:soff + ts, :])
                f_t = fpool.tile([P, ts_max, C], BF16, tag="f")
                nc.sync.dma_start(
                    f_t[:, :ts, :],
                    xeA_d[:, soff * C:(soff + ts) * C]
                    .rearrange("p (t c) -> p t c", c=C))
                y_st = vpool.tile([P, NBMAX, C], BF16, tag="yst")
                for b in range(b0, b0 + nb):
                    kk = b - b0
                    y_ps = ypsum.tile([P, C], F32, tag="ypsum")
                    g0 = int(gA["base_bc"][b, 0] - soff)
                    ntile = int(gA["T_BC"][b, 0])
                    for k in range(ntile):
                        t = g0 + k
                        sel = make_sel(mrow[:, t, 0:1], mrow[:, t, 1:2],
                                       mrow[:, t, 2:3], mrow[:, t, 3:4],
                                       act_every=5, weighted=False)
                        nc.tensor.matmul(
                            out=y_ps[:], lhsT=sel[:], rhs=f_t[:, t, :],
                            start=(k == 0), stop=(k == ntile - 1))
                    nc.vector.tensor_copy(y_st[:, kk, :], y_ps[:])
                nc.sync.dma_start(
                    y1s[b0 * P:(b0 + nb) * P, :]
                    .rearrange("(g p) c -> p g c", p=P),
                    y_st[:, :nb, :])
                for g in ag_after_stage.get(si, []):
                    emit_ag(g)

            # ---------------- dense x0/x1 loops (overlap AG) -------------
            def emit_x0_x1(gsz=4):
                for g0 in range(0, B, gsz):
                    gn = min(gsz, B - g0)
                    xT_t = vpool.tile([C, gsz * P], BF16, tag="xT")
                    nc.sync.dma_start(xT_t[:, :gn * P],
                                      xT_d[:, g0 * P:(g0 + gn) * P])
                    y_ld = vpool.tile([P, gsz, C], BF16, tag="yld")
                    nc.sync.dma_start(
                        y_ld[:, :gn, :],
                        y1s[g0 * P:(g0 + gn) * P, :]
                        .rearrange("(g p) c -> p g c", p=P))
                    x01_st = vpool.tile([P, gsz, 2 * C], F32, tag="x01st")
                    for k in range(gn):
                        b = g0 + k
                        x0_ps = xpsum.tile([P, C], F32, tag="xpsum")
                        nc.tensor.matmul(
                            out=x0_ps[:], lhsT=xT_t[:, k * P:(k + 1) * P],
                            rhs=w0_t, start=True, stop=True)
                        nc.vector.tensor_tensor(
                            out=x01_st[:, k, 0:C], in0=x0_ps[:], in1=b0b,
                            op=mybir.AluOpType.add)
                        yT_ps = tpsum.tile([P, C], BF16, tag="tpsum")
                        nc.tensor.transpose(yT_ps[:], y_ld[:, k, :], eye_t)
                        yT_sb = vpool.tile([P, C], BF16, tag="ytsb")
                        nc.scalar.activation(
                            out=yT_sb[:], in_=yT_ps[:],
                            func=mybir.ActivationFunctionType.Copy,
                            scale=1.0)
                        x1_ps = xpsum.tile([P, C], F32, tag="xpsum")
                        nc.tensor.matmul(out=x1_ps[:], lhsT=yT_sb[:],
                                         rhs=w1_t, start=True, stop=True)
                        tmp1 = zpool.tile([P, C], F32, tag="tmp")
                        nc.scalar.activation(
                            out=tmp1[:], in_=b1b,
                            func=mybir.ActivationFunctionType.Copy,
                            scale=d1col[:, b:b + 1])
                        nc.vector.tensor_tensor(
                            out=x01_st[:, k, C:2 * C], in0=x1_ps[:],
                            in1=tmp1[:], op=mybir.AluOpType.add)
                    nc.sync.dma_start(
                        out_d[g0 * P:(g0 + gn) * P, 0:2 * C]
                        .rearrange("(g p) c -> p g c", p=P),
                        x01_st[:, :gn, :])

            # ---------------- pass B: one sweep per source window ---------
            def emit_x2_blocks(b0, nb):
                x2_st = vpool.tile([P, NBMAX, C], F32, tag="x2st")
                for b in range(b0, b0 + nb):
                    kk = b - b0
                    x_ps = xpsum.tile([P, C], F32, tag="xpsum")
                    nc.tensor.matmul(out=x_ps[:], lhsT=ystg[0][:, b, :],
                                     rhs=w2_t, start=True, stop=False)
                    nc.tensor.matmul(out=x_ps[:], lhsT=ystg[1][:, b, :],
                                     rhs=w2_t, start=False, stop=True)
                    tmp2 = zpool.tile([P, C], F32, tag="tmp")
                    nc.scalar.activation(
                        out=tmp2[:], in_=b2b,
                        func=mybir.ActivationFunctionType.Copy,
                        scale=d2col[:, b:b + 1])
                    nc.vector.tensor_tensor(
                        out=x2_st[:, kk, :], in0=x_ps[:],
                        in1=tmp2[:], op=mybir.AluOpType.add)
                nc.sync.dma_start(
                    out_d[b0 * P:(b0 + nb) * P, 2 * C:3 * C]
                    .rearrange("(g p) c -> p g c", p=P),
                    x2_st[:, :nb, :])

            def emit_passB_sweep(sw):
                clo = 0 if sw == 0 else ch_half
                chi = ch_half if sw == 0 else NG
                last = (sw == 1)
                for si, (b0, nb, soff) in enumerate(stagesB):
                    if sw == 0 and si == min(3, len(stagesB) - 1):
                        for g in range(NG - 1, NG):
                            emit_ag(g)
                    if sw == 0 and si == min(7, len(stagesB) - 1) and NG > 2:
                        emit_ag(NG - 2)
                    lens = gB["call_lens"][si]
                    rel0 = sum(lens[:clo])
                    ts_sw = sum(lens[clo:chi])
                    if ts_sw == 0:
                        if last:
                            emit_x2_blocks(b0, nb)
                        continue
                    so = soff + rel0
                    mrow = mpoolB.tile([P, ts_max, 4], F32, tag="metaB")
                    nc.sync.dma_start(mrow[:, :ts_sw, :],
                                      emetaB_d[:, so:so + ts_sw, :])
                    idx_t = mpoolB.tile([P, (P * ts_max) // 16], I16,
                                        tag="idxB")
                    i16o = (P * so) // 16
                    i16n = (P * ts_sw) // 16
                    nc.sync.dma_start(idx_t[:, :i16n],
                                      eidxB_d[:, i16o:i16o + i16n])
                    f_t = fpoolB.tile([P, ts_max, C], BF16, tag="fB")
                    rel = 0
                    for c in range(clo, chi):
                        tsc = lens[c]
                        if tsc == 0:
                            continue
                        nidx = P * tsc
                        nc.gpsimd.dma_gather(
                            out_ap=f_t[:, rel:rel + tsc, :],
                            in_ap=y1fg[c][:, :],
                            idxs_ap=idx_t[:, (P * rel) // 16:
                                          (P * rel) // 16 + nidx // 16],
                            num_idxs=nidx,
                            num_idxs_reg=nidx,
                            elem_size=C,
                            single_packet=False,
                            queue_num=(si + c) % 4,
                        )
                        rel += tsc
                    for b in range(b0, b0 + nb):
                        tl = [(int(gB["base_bc"][b, c] - so),
                               int(gB["T_BC"][b, c]))
                              for c in range(clo, chi)
                              if gB["T_BC"][b, c] > 0]
                        ntile = sum(n for _, n in tl)
                        if ntile == 0:
                            continue
                        y_ps = ypsum.tile([P, C], F32, tag="ypsum")
                        k = 0
                        for (g0, n) in tl:
                            for t in range(g0, g0 + n):
                                sel = make_sel(mrow[:, t, 0:1],
                                               mrow[:, t, 1:2],
                                               mrow[:, t, 2:3],
                                               mrow[:, t, 3:4],
                                               act_every=SEL_ACT_EVERY)
                                nc.tensor.matmul(
                                    out=y_ps[:], lhsT=f_t[:, t, :],
                                    rhs=sel[:],
                                    start=(k == 0), stop=(k == ntile - 1))
                                k += 1
                        nc.scalar.activation(
                            out=ystg[sw][:, b, :], in_=y_ps[:],
                            func=mybir.ActivationFunctionType.Copy,
                            scale=1.0)
                    if last:
                        emit_x2_blocks(b0, nb)

            emit_x0_x1()
            emit_passB_sweep(0)
            emit_passB_sweep(1)

    nc.compile()
    return nc


# ---------------------------------------------------------------------------
# entry point
# ---------------------------------------------------------------------------

def make_in_maps(meta, W0, b0, W1, b1, W2, b2, n_cores=N_CORES):
    C, S = meta["C"], meta["S"]
    iota = np.tile(np.arange(P, dtype=np.float32), (P, 1)).astype(NPBF)
    eye = np.eye(P, dtype=np.float32).astype(NPBF)
    consts = np.ascontiguousarray(np.concatenate([iota, eye], axis=1))
    wmat = np.concatenate(
        [np.asarray(W0, np.float32), np.asarray(W1, np.float32),
         np.asarray(W2, np.float32)], axis=1).astype(NPBF)
    B = meta["B"]
    b0b = np.tile(np.asarray(b0, np.float32), (P, 1))
    b1b = np.tile(np.asarray(b1, np.float32), (P, 1))
    b2b = np.tile(np.asarray(b2, np.float32), (P, 1))
    in_maps = []
    for c in range(n_cores):
        d1c = np.ascontiguousarray(
            meta["d1"][c].reshape(B, P).T)          # [128, B]
        d2c = np.ascontiguousarray(
            meta["d2"][c].reshape(B, P).T)
        consts32 = np.ascontiguousarray(np.concatenate(
            [b0b, b1b, b2b, d1c, d2c], axis=1))
        in_maps.append({
            "xeA": meta["xeA"][c],
            "emeta": _meta_tile(meta["gA"]["emeta"][c], meta["LT"]),
            "emetaB": _meta_tile(meta["gB"]["emeta"][c], meta["LTB"]),
            "eidxB": meta["eidxB16"][c],
            "xT": meta["xT"][c],
            "wmat": wmat,
            "consts": consts,
            "consts32": consts32,
        })
    return in_maps


def _meta_tile(em, LT_):
    # [L, 4] edge-order -> [128, LT, 4]: edge j -> partition j%128, tile j//128
    return np.ascontiguousarray(em.reshape(LT_, P, 4).transpose(1, 0, 2))


def kernel(x, row, col, edge_weight, W0, b0, W1, b1, W2, b2):
    x = np.asarray(x, np.float32)
    row = np.asarray(row, np.int32)
    col = np.asarray(col, np.int32)
    edge_weight = np.asarray(edge_weight, np.float32)
    N = x.shape[0]

    meta = _prep(x, row, col, edge_weight)
    nc = build_program(meta)
    in_maps = make_in_maps(meta, W0, b0, W1, b1, W2, b2)
    res = run_bass_kernel_spmd(nc, in_maps, core_ids=list(range(N_CORES)))
    out = np.concatenate([r["out"] for r in res.results], axis=0)
    return np.ascontiguousarray(out[:N])


if __name__ == "__main__":
    rng = np.random.default_rng(0)
    N, C, E = 2048, 128, 8192
    x = rng.standard_normal((N, C), dtype=np.float32)
    row = rng.integers(0, N, E).astype(np.int32)
    col = rng.integers(0, N, E).astype(np.int32)
    w = rng.random(E, dtype=np.float32)
    meta = _prep(x, row, col, w)
    print("tiles A:", meta["LT"], "tiles B:", meta["LTB"],
          "stages:", len(meta["gA"]["stages"]), len(meta["gB"]["stages"]))
